# revision 22
# baseline (speedup 1.0000x reference)
"""MetaBaseline (retrieval_knn) Trainium2 kernel.

Computation (per episode b):
  q  = l2norm(input1[b])            # [75, 25, 640] over channel
  s  = l2norm(input2[b])            # [5, 5, 25, 640]
  att = softmax_hw(s @ rpn_w)       # rpn_b is softmax-invariant
  cg  = leaky(sum_hw(att * s))
  feat = mean_shot(mean_hw(s) + 5 * cg)
  sim[b] = mean_hw(q) @ feat.T      # [75, 5]

Sharding: data-parallel over episodes, 4 per core on 8 cores.

Design (v3): bulk data moves and streams as bf16 (PE: 1 cycle/col vs 2
for fp16/fp32r; rel-err budget 2e-2, measured ~3.5e-3). Input DMAs are
SWDGE (gpsimd) — HWDGE 2D descriptor generation caps at ~130GB/s while
SWDGE sustains ~200 — and are issued before any other gpsimd work so the
SDMA engines stream continuously from t=0; the full per-core shard
(~52KB/partition) is preloaded, no buffer recycling. Constants (masks /
identity / broadcast-w) are host-precomputed, one small sync-DMA.
1/sqrt(n2) runs on ACT as exp(-0.5*ln(x)) — square, ln and exp live in
one table set — replacing a 10-op DVE Newton per batch with 2 ACT ops.
Per-slot mask scaling (inv-norm / att weights folded into PE stationary
masks) is batched into a few broadcast-AP DVE multiplies. The support
stream computes the attention-sum and the mean in ONE PE pass
(stationary [125, 57]); feat is produced directly transposed via
fp.T @ shotm; sim is computed as [way, qn] and un-transposed on the
host. The per-episode tail (qm transpose + sim) is software-pipelined
one episode behind the main passes so no engine queue stalls on a
cross-engine round trip.
"""

import os
import sys
from contextlib import ExitStack

sys.path.insert(0, "/opt/trn_rl_repo")

import numpy as np
import ml_dtypes

import concourse.bass as bass
import concourse.tile as tile
from concourse import bacc, mybir
from concourse.bass_utils import run_bass_kernel_spmd

# Pin every activation to the natural_log_exp_and_others table set (it
# holds square, exp AND ln). The default chooser maps each function to
# its "home" set, which thrashes ACT_TABLE_LOADs (~1.3us each) between
# Square and Ln/Exp; one shared set means exactly one load. Indices of
# the other sets are preserved (emptied, not removed) so the emitted
# act_func_set_id still matches act_info.json.
import concourse.bacc as _bacc_mod
from concourse.hw_specs import get_activation_tables as _orig_act_tables

_ACT_SET = "natural_log_exp_and_others"


def _pinned_act_tables(arch):
    return {k: (v if k == _ACT_SET else set())
            for k, v in _orig_act_tables(arch).items()}


_bacc_mod.get_activation_tables = _pinned_act_tables

F32 = mybir.dt.float32
BF = mybir.dt.bfloat16
AX = mybir.AxisListType
OP = mybir.AluOpType
AF = mybir.ActivationFunctionType

# Problem constants (fixed by the problem statement).
B, QN, WAY, SHOT, HH, WW, C = 32, 75, 5, 5, 5, 5, 640
NCORES = 8
E = B // NCORES        # 4 episodes per core
HW = HH * WW           # 25 spatial positions
QD = QN * HW           # 1875 query descriptors / episode
SD = WAY * SHOT * HW   # 625 support descriptors / episode
P = 125                # descriptors per tile
QT = QD // P           # 15 query slots / episode (desc d = 15p + j)
ST = SD // P           # 5 support slots / episode (desc d = 5p + j)
NMAP = WAY * SHOT      # 25 support maps / episode
NCH = 3                # q DMA chunks (5 slots each)
SPC = QT // NCH        # slots per chunk
GAMMA = 5.0
SLOPE = 0.01
CH = C // 2            # 320-column halves (one PSUM bank each)
MB = 32                # mean-row base partition in the fused support psum
SW = MB + NMAP         # fused stationary width (57)
QNP = QN + 1           # padded transpose chunk stride (PSUM 4B align)

# constants tensor layout (free-axis offsets, bf16)
QM0 = 0                    # qmasks  [125, 15*75], value 1/25
SM0 = QM0 + QT * QN        # smasks  [125, 5*25],  value 1.0 (sums + att)
SM2 = SM0 + ST * NMAP      # smasks  [125, 5*25],  value 1/25 (hw-mean)
WB0 = SM2 + ST * NMAP      # w bcast [128, 640]
ID0 = WB0 + C              # identity [75, 75]
SH0 = ID0 + QN             # shotm   [25, 5], value 1/5
CW = SH0 + WAY             # = 2095

# engine split of the per-slot norm passes (True -> ACT)
S_ACT = (True, True, True, False, False)
Q_ACT = (True, True, True, True, False,
         True, True, True, False, False,
         True, True, False, False, False)


def _build_body(ctx: ExitStack, tc: "tile.TileContext", i1, i2, cst, out):
    nc = tc.nc

    cpool = ctx.enter_context(tc.tile_pool(name="consts", bufs=1))
    dpool = ctx.enter_context(tc.tile_pool(name="data", bufs=1))
    scr_pool = ctx.enter_context(tc.tile_pool(name="scratch", bufs=1))
    stats = ctx.enter_context(tc.tile_pool(name="stats", bufs=2))
    sel_pool = ctx.enter_context(tc.tile_pool(name="sel", bufs=2))
    sb_pool = ctx.enter_context(tc.tile_pool(name="sbwork", bufs=2))
    ps = ctx.enter_context(tc.tile_pool(name="ps", bufs=1, space="PSUM"))

    # ---- all input DMAs first (SWDGE; gpsimd queue head) ----
    # s_sl[e][j] is the [125, 640] slot view; episode 0's support tensor is
    # loaded as 5 per-slot DMAs so the very first compute tile lands ~10us
    # earlier (a big DMA's completion waits on all 125 descriptors).
    s_sl, q_t = [], []
    for e in range(E):
        if e == 0:
            sl = []
            for j in range(ST):
                t_ = dpool.tile([P, C], BF, name=f"s0_{j}", tag=f"s0_{j}")
                nc.gpsimd.dma_start(t_[:], i2[0, :, C * j:C * (j + 1)])
                sl.append(t_[:])
        else:
            st_ = dpool.tile([P, ST * C], BF, name=f"s_{e}", tag=f"s_{e}")
            nc.gpsimd.dma_start(st_[:], i2[e])
            sl = [st_[:, C * j:C * (j + 1)] for j in range(ST)]
        qc = []
        for c in range(NCH):
            qt_ = dpool.tile([P, SPC * C], BF, name=f"q_{e}_{c}",
                             tag=f"q_{e}_{c}")
            nc.gpsimd.dma_start(qt_[:], i1[e, :, SPC * C * c:SPC * C * (c + 1)])
            qc.append(qt_)
        s_sl.append(sl)
        q_t.append(qc)

    # ---- constants (host-precomputed, one sync DMA) ----
    consts = cpool.tile([128, CW], BF, name="consts")
    nc.sync.dma_start(consts[:], cst)
    smask = [consts[0:P, SM0 + NMAP * j:SM0 + NMAP * (j + 1)] for j in range(ST)]
    smask3 = consts[0:P, SM0:SM0 + ST * NMAP].rearrange(
        "p (j m) -> p j m", j=ST)
    smask3m = consts[0:P, SM2:SM2 + ST * NMAP].rearrange(
        "p (j m) -> p j m", j=ST)
    qmask3 = [consts[0:P, QM0 + SPC * QN * c:QM0 + SPC * QN * (c + 1)]
              .rearrange("p (j q) -> p j q", j=SPC) for c in range(NCH)]
    wbc = consts[0:P, WB0:WB0 + C]
    ident = consts[0:QN, ID0:ID0 + QN]
    shotm = consts[0:NMAP, SH0:SH0 + WAY]

    # fused support stationary [125, ST, 57] (cols 25-31 stay zero forever)
    st_all = cpool.tile([P, ST, SW], BF, name="st_all")
    nc.vector.memset(st_all[:, :, NMAP:MB], 0.0)

    def slot(big, j):
        return big[:, C * j:C * (j + 1)]

    def pe_tickle(anchor_ap):
        """Dummy 1-column LDWEIGHTS anchored to a freshly-written bf16
        tile. Executes in ~85ns as soon as the anchor is ready, keeping
        the PE HAM activity monitor from re-throttling the clock to
        1.2GHz during ACT/DVE-bound phases (idle windows >3.4us drop the
        PE to K=4/8). All real matmuls self-load weights, so clobbering
        the stationary register is safe."""
        nc.tensor.ldweights(anchor_ap)

    def rsqrt_act(dst, x, n, tag):
        """dst = 1/sqrt(x) on ACT: exp(-0.5*ln(x)); same table set as
        Square/Exp, so no ACT_TABLE_LOAD switches."""
        t = stats.tile([P, n], F32, name=f"rs_{tag}", tag=f"rs_{tag}")
        nc.scalar.activation(t[:], x, AF.Ln)
        nc.scalar.activation(dst, t[:], AF.Exp, scale=-0.5)

    def norm_pass(sl, acc_col, on_act):
        if on_act:
            scr = scr_pool.tile([P, C], BF, name="sq_a", tag="sq_a")
            nc.scalar.activation(scr[:], sl, AF.Square, accum_out=acc_col)
        else:
            scr = scr_pool.tile([P, C], BF, name="sq_v", tag="sq_v")
            nc.vector.scalar_tensor_tensor(
                out=scr[:], in0=sl, scalar=1.0, in1=sl,
                op0=OP.mult, op1=OP.mult, accum_out=acc_col)
        return scr

    # per-episode state carried into the pipelined tail
    qm_sb_t, ftT_t, tq_t, qmT_t = [None] * E, [None] * E, [None] * E, [None] * E

    def emit_tail_a(e):
        """PE transpose of qm (needs qm_sb[e]), on the prior episode's
        psum bank."""
        tq_ps = ps.tile([128, WAY * QNP], BF, name=f"tq_{e}", tag="tq")
        for cc in range(WAY):
            nc.tensor.transpose(tq_ps[:, QNP * cc:QNP * cc + QN],
                                qm_sb_t[e][:, 128 * cc:128 * (cc + 1)], ident)
        tq_t[e] = tq_ps
        qmT = sb_pool.tile([128, WAY * QNP], BF, name=f"qmT_{e}", tag="qmT")
        nc.scalar.copy(qmT[:], tq_ps[:])
        qmT_t[e] = qmT

    def emit_tail_b(e):
        sim_ps = ps.tile([WAY, QN], F32, name=f"sim_{e}", tag="sim")
        for cc in range(WAY):
            nc.tensor.matmul(sim_ps[:], ftT_t[e][:, WAY * cc:WAY * (cc + 1)],
                             qmT_t[e][:, QNP * cc:QNP * cc + QN],
                             start=(cc == 0), stop=(cc == WAY - 1))
        sim_sb = sb_pool.tile([WAY, QN], F32, name=f"sim_sb_{e}", tag="sim_sb")
        nc.vector.tensor_copy(sim_sb[:], sim_ps[:])
        nc.sync.dma_start(out[e], sim_sb[:])

    for e in range(E):
        ssl = s_sl[e]
        # ================= support side =================
        sn2 = stats.tile([P, ST], F32, name=f"sn2_{e}", tag="sn2")
        rr = stats.tile([P, ST], F32, name=f"rr_{e}", tag="rr")
        # DVE s-norms first so ACT's rsqrt isn't stuck behind the logits
        for j in range(ST):
            if not S_ACT[j]:
                norm_pass(ssl[j], sn2[:, j:j + 1], False)
        for j in range(ST):
            if S_ACT[j]:
                scr = norm_pass(ssl[j], sn2[:, j:j + 1], True)
                pe_tickle(scr[:, 0:1])
        for j in range(ST):
            scr2 = scr_pool.tile([P, C], BF, name="s_tt", tag="s_tt")
            nc.vector.scalar_tensor_tensor(
                out=scr2[:], in0=ssl[j], scalar=1.0, in1=wbc,
                op0=OP.mult, op1=OP.mult, accum_out=rr[:, j:j + 1])
        sinv = stats.tile([P, ST], BF, name=f"sinv_{e}", tag="sinv")
        rsqrt_act(sinv[:], sn2[:], ST, f"s{e % 2}")
        # softmax over hw within each map (logits tiny: no max-shift)
        lg = stats.tile([P, ST], F32, name=f"lg_{e}", tag="lg")
        nc.vector.tensor_mul(lg[:], rr[:], sinv[:])
        el = stats.tile([P, ST], BF, name=f"el_{e}", tag="el")
        nc.scalar.activation(el[:], lg[:], AF.Exp)
        # per-map sums of exp -> softmax reciprocal
        sums = ps.tile([NMAP, 1], F32, name=f"sums_{e}", tag="sums")
        for j in range(ST):
            nc.tensor.matmul(sums[:], smask[j], el[:, j:j + 1],
                             start=(j == 0), stop=(j == ST - 1))
        rec = stats.tile([NMAP, 1], F32, name=f"rec_{e}", tag="rec")
        nc.vector.reciprocal(rec[:], sums[:])
        # unnormalized att weights (softmax recip applied at cg evacuation)
        uw = stats.tile([P, ST], BF, name=f"uw_{e}", tag="uw")
        nc.vector.tensor_mul(uw[:], el[:], sinv[:])
        # batched stationary builds (pad cols stay zero)
        nc.vector.tensor_mul(st_all[:, :, 0:NMAP], smask3,
                             uw[:].broadcast_to((P, ST, NMAP)))
        nc.vector.tensor_mul(st_all[:, :, MB:SW], smask3m,
                             sinv[:].broadcast_to((P, ST, NMAP)))
        cg_ps = [ps.tile([SW, CH], F32, name=f"cg{h}_{e}", tag=f"cg{h}")
                 for h in range(2)]
        for j in range(ST):
            for h in range(2):
                nc.tensor.matmul(cg_ps[h][:], st_all[:, j, :],
                                 ssl[j][:, CH * h:CH * (h + 1)],
                                 start=(j == 0), stop=(j == ST - 1))
        # evacuate: cg rows 0-24 (x softmax recip), mean rows MB..MB+24
        cg_sb = sb_pool.tile([NMAP, C], F32, name=f"cg_sb_{e}", tag="cg_sb")
        for h in range(2):
            nc.vector.tensor_scalar_mul(cg_sb[:, CH * h:CH * (h + 1)],
                                        cg_ps[h][0:NMAP, :], rec[:, 0:1])
        lk = sb_pool.tile([NMAP, C], F32, name=f"lk_{e}", tag="lk")
        nc.vector.scalar_tensor_tensor(
            out=lk[:], in0=cg_sb[:], scalar=SLOPE, in1=cg_sb[:],
            op0=OP.mult, op1=OP.max)
        fp = sb_pool.tile([NMAP, C], BF, name=f"fp_{e}", tag="fp")
        for h in range(2):
            nc.vector.scalar_tensor_tensor(
                out=fp[:, CH * h:CH * (h + 1)], in0=lk[:, CH * h:CH * (h + 1)],
                scalar=GAMMA, in1=cg_ps[h][MB:MB + NMAP, :],
                op0=OP.mult, op1=OP.add)
        # featT[c, w] directly: fp.T @ shotm, chunked over c
        ftT_ps = ps.tile([128, WAY * WAY], F32, name=f"ftT_{e}", tag="ftT")
        for cc in range(WAY):
            nc.tensor.matmul(ftT_ps[:, WAY * cc:WAY * (cc + 1)],
                             fp[:, 128 * cc:128 * (cc + 1)], shotm)
        ftT = sb_pool.tile([128, WAY * WAY], BF, name=f"ftTs_{e}", tag="ftTs")
        nc.vector.tensor_copy(ftT[:], ftT_ps[:])
        ftT_t[e] = ftT

        # ================= query side =================
        qn2 = stats.tile([P, QT], F32, name=f"qn2_{e}", tag="qn2")
        qinv = stats.tile([P, QT], BF, name=f"qinv_{e}", tag="qinv")
        qm = [ps.tile([QN, CH], F32, name=f"qm{h}_{e}", tag=f"qm{h}")
              for h in range(2)]
        for c in range(NCH):
            for jj in range(SPC):
                j = SPC * c + jj
                scr = norm_pass(slot(q_t[e][c], jj), qn2[:, j:j + 1], Q_ACT[j])
                if Q_ACT[j] and jj % 2 == 1:
                    pe_tickle(scr[:, 0:1])
            lo = SPC * c
            rsqrt_act(qinv[:, lo:lo + SPC], qn2[:, lo:lo + SPC], SPC,
                      f"q{c}_{e % 2}")
            sel = sel_pool.tile([P, SPC, QN], BF, name=f"sel{c}",
                                tag=f"sel{c}")
            nc.vector.tensor_mul(sel[:], qmask3[c],
                                 qinv[:, lo:lo + SPC]
                                 .broadcast_to((P, SPC, QN)))
            for jj in range(SPC):
                j = lo + jj
                for h in range(2):
                    nc.tensor.matmul(
                        qm[h][:], sel[:, jj, :],
                        slot(q_t[e][c], jj)[:, CH * h:CH * (h + 1)],
                        start=(j == 0), stop=(j == QT - 1))
            # spread the previous episode's tail into the q-norm gaps
            if e > 0:
                if c == 0:
                    emit_tail_a(e - 1)
                elif c == 1:
                    emit_tail_b(e - 1)
        qm_sb = sb_pool.tile([QN, C], BF, name=f"qm_sb_{e}", tag="qm_sb")
        for h in range(2):
            nc.scalar.copy(qm_sb[:, CH * h:CH * (h + 1)], qm[h][:])
        qm_sb_t[e] = qm_sb
    emit_tail_a(E - 1)
    emit_tail_b(E - 1)


def build_program():
    nc = bacc.Bacc("TRN2", target_bir_lowering=False, debug=False,
                   num_devices=NCORES)
    inp1 = nc.dram_tensor("input1", [E, P, QT * C], BF, kind="ExternalInput")
    inp2 = nc.dram_tensor("input2", [E, P, ST * C], BF, kind="ExternalInput")
    cst = nc.dram_tensor("consts", [128, CW], BF, kind="ExternalInput")
    out = nc.dram_tensor("sim", [E, WAY, QN], F32, kind="ExternalOutput")
    with tile.TileContext(nc) as tc, ExitStack() as ctx:
        _build_body(ctx, tc, inp1.ap(), inp2.ap(), cst.ap(), out.ap())
    nc.compile()
    return nc


_NC = None


def _get_nc():
    global _NC
    if _NC is None:
        _NC = build_program()
    return _NC


def _build_consts(rpn_w):
    cst = np.zeros((128, CW), np.float32)
    # qmask: descriptor d = 15p + j belongs to query q = d // 25
    pp = np.arange(P)
    for j in range(QT):
        cst[pp, QM0 + QN * j + (15 * pp + j) // HW] = 1.0 / HW
    for j in range(ST):
        cst[pp, SM0 + NMAP * j + (5 * pp + j) // HW] = 1.0
        cst[pp, SM2 + NMAP * j + (5 * pp + j) // HW] = 1.0 / HW
    cst[:, WB0:WB0 + C] = np.asarray(rpn_w, np.float32).reshape(1, C)
    cst[np.arange(QN), ID0 + np.arange(QN)] = 1.0
    m = np.arange(NMAP)
    cst[m, SH0 + m // SHOT] = 1.0 / SHOT
    return cst.astype(ml_dtypes.bfloat16)


def shard_inputs(input1, input2, rpn_w, rpn_b=None):
    """Shard over episodes; [E, 1875, 640] -> [E, 125, 15*640] is a pure
    reshape (descriptor d = 15p + j, slots consecutive in DRAM)."""
    i1 = np.asarray(input1, np.float32).reshape(B, P, QT * C).astype(
        ml_dtypes.bfloat16)
    i2 = np.asarray(input2, np.float32).reshape(B, P, ST * C).astype(
        ml_dtypes.bfloat16)
    cst = _build_consts(rpn_w)
    in_maps = []
    for i in range(NCORES):
        in_maps.append({
            "input1": np.ascontiguousarray(i1[E * i:E * (i + 1)]),
            "input2": np.ascontiguousarray(i2[E * i:E * (i + 1)]),
            "consts": cst,
        })
    return in_maps


def _ensure_ntff_hook():
    """Install the NTFF profile hook (the image's antenv lacks axon_hooks)."""
    import types
    import antenv

    if "antenv.axon_hooks" not in sys.modules:
        mod = types.ModuleType("antenv.axon_hooks")
        mod._hook = None
        mod.set_axon_ntff_profile_hook = lambda h: setattr(mod, "_hook", h)
        mod.get_axon_ntff_profile_hook = lambda: mod._hook
        sys.modules["antenv.axon_hooks"] = mod
        antenv.axon_hooks = mod
    mod = sys.modules["antenv.axon_hooks"]
    if mod.get_axon_ntff_profile_hook() is None:
        from trn_agent_boot.trn_boot import _ntff_profile_via_ctypes
        hook = _ntff_profile_via_ctypes("/opt/axon/libaxon_pjrt.so")
        if hook is not None:
            mod.set_axon_ntff_profile_hook(hook)


def kernel(input1, input2, rpn_w, rpn_b=None, **run_kwargs):
    if run_kwargs.get("trace"):
        _ensure_ntff_hook()
    nc = _get_nc()
    in_maps = shard_inputs(input1, input2, rpn_w)
    res = run_bass_kernel_spmd(nc, in_maps, list(range(NCORES)), **run_kwargs)
    # sim comes back [E, way, qn]; un-transpose on the host
    out = np.concatenate(
        [np.transpose(r["sim"], (0, 2, 1)) for r in res.results], axis=0)
    if run_kwargs:
        kernel.last_results = res
    return out.astype(np.float32)


# revision 27
# speedup vs baseline: 1.3064x; 1.3064x over previous
"""MetaBaseline (retrieval_knn) Trainium2 kernel.

Computation (per episode b):
  q  = l2norm(input1[b])            # [75, 25, 640] over channel
  s  = l2norm(input2[b])            # [5, 5, 25, 640]
  att = softmax_hw(s @ rpn_w)       # rpn_b is softmax-invariant
  cg  = leaky(sum_hw(att * s))
  feat = mean_shot(mean_hw(s) + 5 * cg)
  sim[b] = mean_hw(q) @ feat.T      # [75, 5]

Sharding: data-parallel over episodes, 4 per core on 8 cores.

Design (v3): bulk data moves and streams as bf16 (PE: 1 cycle/col vs 2
for fp16/fp32r; rel-err budget 2e-2, measured ~3.5e-3). Input DMAs are
SWDGE (gpsimd) — HWDGE 2D descriptor generation caps at ~130GB/s while
SWDGE sustains ~200 — and are issued before any other gpsimd work so the
SDMA engines stream continuously from t=0; the full per-core shard
(~52KB/partition) is preloaded, no buffer recycling. Constants (masks /
identity / broadcast-w) are host-precomputed, one small sync-DMA.
1/sqrt(n2) runs on ACT as exp(-0.5*ln(x)) — square, ln and exp live in
one table set — replacing a 10-op DVE Newton per batch with 2 ACT ops.
Per-slot mask scaling (inv-norm / att weights folded into PE stationary
masks) is batched into a few broadcast-AP DVE multiplies. The support
stream computes the attention-sum and the mean in ONE PE pass
(stationary [125, 57]); feat is produced directly transposed via
fp.T @ shotm; sim is computed as [way, qn] and un-transposed on the
host. The per-episode tail (qm transpose + sim) is software-pipelined
one episode behind the main passes so no engine queue stalls on a
cross-engine round trip.
"""

import os
import sys
from contextlib import ExitStack

sys.path.insert(0, "/opt/trn_rl_repo")

import numpy as np
import ml_dtypes

import concourse.bass as bass
import concourse.tile as tile
from concourse import bacc, mybir
from concourse.bass_utils import run_bass_kernel_spmd

# Pin every activation to the natural_log_exp_and_others table set (it
# holds square, exp AND ln). The default chooser maps each function to
# its "home" set, which thrashes ACT_TABLE_LOADs (~1.3us each) between
# Square and Ln/Exp; one shared set means exactly one load. Indices of
# the other sets are preserved (emptied, not removed) so the emitted
# act_func_set_id still matches act_info.json.
import concourse.bacc as _bacc_mod
from concourse.hw_specs import get_activation_tables as _orig_act_tables

_ACT_SET = "natural_log_exp_and_others"


def _pinned_act_tables(arch):
    return {k: (v if k == _ACT_SET else set())
            for k, v in _orig_act_tables(arch).items()}


_bacc_mod.get_activation_tables = _pinned_act_tables

F32 = mybir.dt.float32
BF = mybir.dt.bfloat16
AX = mybir.AxisListType
OP = mybir.AluOpType
AF = mybir.ActivationFunctionType

# Problem constants (fixed by the problem statement).
B, QN, WAY, SHOT, HH, WW, C = 32, 75, 5, 5, 5, 5, 640
NCORES = 8
E = B // NCORES        # 4 episodes per core
HW = HH * WW           # 25 spatial positions
QD = QN * HW           # 1875 query descriptors / episode
SD = WAY * SHOT * HW   # 625 support descriptors / episode
P = 125                # descriptors per tile
QT = QD // P           # 15 query slots / episode (desc d = 15p + j)
ST = SD // P           # 5 support slots / episode (desc d = 5p + j)
NMAP = WAY * SHOT      # 25 support maps / episode
NCH = 3                # q DMA chunks (5 slots each)
SPC = QT // NCH        # slots per chunk
GAMMA = 5.0
SLOPE = 0.01
CH = C // 2            # 320-column halves (one PSUM bank each)
MB = 32                # mean-row base partition in the fused support psum
SW = MB + NMAP         # fused stationary width (57)
QNP = QN + 1           # padded transpose chunk stride (PSUM 4B align)

# constants tensor layout (free-axis offsets, bf16)
QM0 = 0                    # qmasks  [125, 15*75], value 1/25
SM0 = QM0 + QT * QN        # smasks  [125, 5*25],  value 1.0 (sums + att)
SM2 = SM0 + ST * NMAP      # smasks  [125, 5*25],  value 1/25 (hw-mean)
WB0 = SM2 + ST * NMAP      # w bcast [128, 640]
ID0 = WB0 + C              # identity [75, 75]
SH0 = ID0 + QN             # shotm   [25, 5], value 1/5
CW = SH0 + WAY             # = 2095

# engine split of the per-slot norm passes (True -> ACT)
S_ACT = (True, True, True, False, False)
Q_ACT = (True, True, True, True, False,
         True, True, True, False, False,
         True, True, False, False, False)


def _build_body(ctx: ExitStack, tc: "tile.TileContext", i1, i2, cst, out):
    nc = tc.nc

    cpool = ctx.enter_context(tc.tile_pool(name="consts", bufs=1))
    dpool = ctx.enter_context(tc.tile_pool(name="data", bufs=1))
    scr_pool = ctx.enter_context(tc.tile_pool(name="scratch", bufs=1))
    stats = ctx.enter_context(tc.tile_pool(name="stats", bufs=2))
    sel_pool = ctx.enter_context(tc.tile_pool(name="sel", bufs=2))
    sb_pool = ctx.enter_context(tc.tile_pool(name="sbwork", bufs=2))
    ps = ctx.enter_context(tc.tile_pool(name="ps", bufs=1, space="PSUM"))

    # ---- all input DMAs first (SWDGE; gpsimd queue head) ----
    # s_sl[e][j] is the [125, 640] slot view; episode 0's support tensor is
    # loaded as 5 per-slot DMAs so the very first compute tile lands ~10us
    # earlier (a big DMA's completion waits on all 125 descriptors).
    s_sl, q_t = [], []
    for e in range(E):
        if e == 0:
            sl = []
            for j in range(ST):
                t_ = dpool.tile([P, C], BF, name=f"s0_{j}", tag=f"s0_{j}")
                nc.gpsimd.dma_start(t_[:], i2[0, :, C * j:C * (j + 1)])
                sl.append(t_[:])
        else:
            st_ = dpool.tile([P, ST * C], BF, name=f"s_{e}", tag=f"s_{e}")
            nc.gpsimd.dma_start(st_[:], i2[e])
            sl = [st_[:, C * j:C * (j + 1)] for j in range(ST)]
        qc = []
        for c in range(NCH):
            qt_ = dpool.tile([P, SPC * C], BF, name=f"q_{e}_{c}",
                             tag=f"q_{e}_{c}")
            nc.gpsimd.dma_start(qt_[:], i1[e, :, SPC * C * c:SPC * C * (c + 1)])
            qc.append(qt_)
        s_sl.append(sl)
        q_t.append(qc)

    # ---- constants (host-precomputed, one sync DMA) ----
    consts = cpool.tile([128, CW], BF, name="consts")
    nc.sync.dma_start(consts[:], cst)
    smask = [consts[0:P, SM0 + NMAP * j:SM0 + NMAP * (j + 1)] for j in range(ST)]
    smask3 = consts[0:P, SM0:SM0 + ST * NMAP].rearrange(
        "p (j m) -> p j m", j=ST)
    smask3m = consts[0:P, SM2:SM2 + ST * NMAP].rearrange(
        "p (j m) -> p j m", j=ST)
    qmask3 = [consts[0:P, QM0 + SPC * QN * c:QM0 + SPC * QN * (c + 1)]
              .rearrange("p (j q) -> p j q", j=SPC) for c in range(NCH)]
    wbc = consts[0:P, WB0:WB0 + C]
    ident = consts[0:QN, ID0:ID0 + QN]
    shotm = consts[0:NMAP, SH0:SH0 + WAY]

    # fused support stationary [125, ST, 57] (cols 25-31 stay zero forever)
    st_all = cpool.tile([P, ST, SW], BF, name="st_all")
    nc.vector.memset(st_all[:, :, NMAP:MB], 0.0)

    def slot(big, j):
        return big[:, C * j:C * (j + 1)]

    def pe_tickle(anchor_ap):
        """Dummy 1-column LDWEIGHTS anchored to a freshly-written bf16
        tile. Executes in ~85ns as soon as the anchor is ready, keeping
        the PE HAM activity monitor from re-throttling the clock to
        1.2GHz during ACT/DVE-bound phases (idle windows >3.4us drop the
        PE to K=4/8). All real matmuls self-load weights, so clobbering
        the stationary register is safe."""
        nc.tensor.ldweights(anchor_ap)

    def rsqrt_act(dst, x, n, tag):
        """dst = 1/sqrt(x) on ACT: exp(-0.5*ln(x)); same table set as
        Square/Exp, so no ACT_TABLE_LOAD switches."""
        t = stats.tile([P, n], F32, name=f"rs_{tag}", tag=f"rs_{tag}")
        nc.scalar.activation(t[:], x, AF.Ln)
        nc.scalar.activation(dst, t[:], AF.Exp, scale=-0.5)

    def norm_pass(sl, acc_col, on_act):
        if on_act:
            scr = scr_pool.tile([P, C], BF, name="sq_a", tag="sq_a")
            nc.scalar.activation(scr[:], sl, AF.Square, accum_out=acc_col)
        else:
            scr = scr_pool.tile([P, C], BF, name="sq_v", tag="sq_v")
            nc.vector.scalar_tensor_tensor(
                out=scr[:], in0=sl, scalar=1.0, in1=sl,
                op0=OP.mult, op1=OP.mult, accum_out=acc_col)
        return scr

    # per-episode state carried into the pipelined tail
    qm_sb_t, ftT_t, tq_t, qmT_t = [None] * E, [None] * E, [None] * E, [None] * E

    def emit_tail_a(e):
        """PE transpose of qm (needs qm_sb[e]), on the prior episode's
        psum bank."""
        tq_ps = ps.tile([128, WAY * QNP], BF, name=f"tq_{e}", tag="tq")
        for cc in range(WAY):
            nc.tensor.transpose(tq_ps[:, QNP * cc:QNP * cc + QN],
                                qm_sb_t[e][:, 128 * cc:128 * (cc + 1)], ident)
        tq_t[e] = tq_ps
        qmT = sb_pool.tile([128, WAY * QNP], BF, name=f"qmT_{e}", tag="qmT")
        nc.scalar.copy(qmT[:], tq_ps[:])
        qmT_t[e] = qmT

    def emit_tail_b(e):
        sim_ps = ps.tile([WAY, QN], F32, name=f"sim_{e}", tag="sim")
        for cc in range(WAY):
            nc.tensor.matmul(sim_ps[:], ftT_t[e][:, WAY * cc:WAY * (cc + 1)],
                             qmT_t[e][:, QNP * cc:QNP * cc + QN],
                             start=(cc == 0), stop=(cc == WAY - 1))
        sim_sb = sb_pool.tile([WAY, QN], F32, name=f"sim_sb_{e}", tag="sim_sb")
        nc.vector.tensor_copy(sim_sb[:], sim_ps[:])
        nc.sync.dma_start(out[e], sim_sb[:])

    for e in range(E):
        ssl = s_sl[e]
        # ================= support side =================
        sn2 = stats.tile([P, ST], F32, name=f"sn2_{e}", tag="sn2")
        rr = stats.tile([P, ST], F32, name=f"rr_{e}", tag="rr")
        # DVE s-norms first so ACT's rsqrt isn't stuck behind the logits
        for j in range(ST):
            if not S_ACT[j]:
                norm_pass(ssl[j], sn2[:, j:j + 1], False)
        for j in range(ST):
            if S_ACT[j]:
                norm_pass(ssl[j], sn2[:, j:j + 1], True)
        for j in range(ST):
            scr2 = scr_pool.tile([P, C], BF, name="s_tt", tag="s_tt")
            nc.vector.scalar_tensor_tensor(
                out=scr2[:], in0=ssl[j], scalar=1.0, in1=wbc,
                op0=OP.mult, op1=OP.mult, accum_out=rr[:, j:j + 1])
        sinv = stats.tile([P, ST], BF, name=f"sinv_{e}", tag="sinv")
        rsqrt_act(sinv[:], sn2[:], ST, f"s{e % 2}")
        # softmax over hw within each map (logits tiny: no max-shift)
        lg = stats.tile([P, ST], F32, name=f"lg_{e}", tag="lg")
        nc.vector.tensor_mul(lg[:], rr[:], sinv[:])
        el = stats.tile([P, ST], BF, name=f"el_{e}", tag="el")
        nc.scalar.activation(el[:], lg[:], AF.Exp)
        # per-map sums of exp -> softmax reciprocal
        sums = ps.tile([NMAP, 1], F32, name=f"sums_{e}", tag="sums")
        for j in range(ST):
            nc.tensor.matmul(sums[:], smask[j], el[:, j:j + 1],
                             start=(j == 0), stop=(j == ST - 1))
        rec = stats.tile([NMAP, 1], F32, name=f"rec_{e}", tag="rec")
        nc.vector.reciprocal(rec[:], sums[:])
        # unnormalized att weights (softmax recip applied at cg evacuation)
        uw = stats.tile([P, ST], BF, name=f"uw_{e}", tag="uw")
        nc.vector.tensor_mul(uw[:], el[:], sinv[:])
        # batched stationary builds (pad cols stay zero)
        nc.vector.tensor_mul(st_all[:, :, 0:NMAP], smask3,
                             uw[:].broadcast_to((P, ST, NMAP)))
        nc.vector.tensor_mul(st_all[:, :, MB:SW], smask3m,
                             sinv[:].broadcast_to((P, ST, NMAP)))
        cg_ps = [ps.tile([SW, CH], F32, name=f"cg{h}_{e}", tag=f"cg{h}")
                 for h in range(2)]
        for j in range(ST):
            for h in range(2):
                nc.tensor.matmul(cg_ps[h][:], st_all[:, j, :],
                                 ssl[j][:, CH * h:CH * (h + 1)],
                                 start=(j == 0), stop=(j == ST - 1))
        # evacuate: cg rows 0-24 (x softmax recip), mean rows MB..MB+24
        cg_sb = sb_pool.tile([NMAP, C], F32, name=f"cg_sb_{e}", tag="cg_sb")
        for h in range(2):
            nc.vector.tensor_scalar_mul(cg_sb[:, CH * h:CH * (h + 1)],
                                        cg_ps[h][0:NMAP, :], rec[:, 0:1])
        lk = sb_pool.tile([NMAP, C], F32, name=f"lk_{e}", tag="lk")
        nc.vector.scalar_tensor_tensor(
            out=lk[:], in0=cg_sb[:], scalar=SLOPE, in1=cg_sb[:],
            op0=OP.mult, op1=OP.max)
        fp = sb_pool.tile([NMAP, C], BF, name=f"fp_{e}", tag="fp")
        for h in range(2):
            nc.vector.scalar_tensor_tensor(
                out=fp[:, CH * h:CH * (h + 1)], in0=lk[:, CH * h:CH * (h + 1)],
                scalar=GAMMA, in1=cg_ps[h][MB:MB + NMAP, :],
                op0=OP.mult, op1=OP.add)
        # featT[c, w] directly: fp.T @ shotm, chunked over c
        ftT_ps = ps.tile([128, WAY * WAY], F32, name=f"ftT_{e}", tag="ftT")
        for cc in range(WAY):
            nc.tensor.matmul(ftT_ps[:, WAY * cc:WAY * (cc + 1)],
                             fp[:, 128 * cc:128 * (cc + 1)], shotm)
        ftT = sb_pool.tile([128, WAY * WAY], BF, name=f"ftTs_{e}", tag="ftTs")
        nc.vector.tensor_copy(ftT[:], ftT_ps[:])
        ftT_t[e] = ftT

        # ================= query side =================
        qn2 = stats.tile([P, QT], F32, name=f"qn2_{e}", tag="qn2")
        qinv = stats.tile([P, QT], BF, name=f"qinv_{e}", tag="qinv")
        qm = [ps.tile([QN, CH], F32, name=f"qm{h}_{e}", tag=f"qm{h}")
              for h in range(2)]
        for c in range(NCH):
            for jj in range(SPC):
                j = SPC * c + jj
                norm_pass(slot(q_t[e][c], jj), qn2[:, j:j + 1], Q_ACT[j])
            lo = SPC * c
            rsqrt_act(qinv[:, lo:lo + SPC], qn2[:, lo:lo + SPC], SPC,
                      f"q{c}_{e % 2}")
            sel = sel_pool.tile([P, SPC, QN], BF, name=f"sel{c}",
                                tag=f"sel{c}")
            nc.vector.tensor_mul(sel[:], qmask3[c],
                                 qinv[:, lo:lo + SPC]
                                 .broadcast_to((P, SPC, QN)))
            for jj in range(SPC):
                j = lo + jj
                for h in range(2):
                    nc.tensor.matmul(
                        qm[h][:], sel[:, jj, :],
                        slot(q_t[e][c], jj)[:, CH * h:CH * (h + 1)],
                        start=(j == 0), stop=(j == QT - 1))
            # spread the previous episode's tail into the q-norm gaps
            if e > 0:
                if c == 0:
                    emit_tail_a(e - 1)
                elif c == 1:
                    emit_tail_b(e - 1)
        qm_sb = sb_pool.tile([QN, C], BF, name=f"qm_sb_{e}", tag="qm_sb")
        for h in range(2):
            nc.scalar.copy(qm_sb[:, CH * h:CH * (h + 1)], qm[h][:])
        qm_sb_t[e] = qm_sb
    emit_tail_a(E - 1)
    emit_tail_b(E - 1)


def build_program():
    nc = bacc.Bacc("TRN2", target_bir_lowering=False, debug=False,
                   num_devices=NCORES)
    inp1 = nc.dram_tensor("input1", [E, P, QT * C], BF, kind="ExternalInput")
    inp2 = nc.dram_tensor("input2", [E, P, ST * C], BF, kind="ExternalInput")
    cst = nc.dram_tensor("consts", [128, CW], BF, kind="ExternalInput")
    out = nc.dram_tensor("sim", [E, WAY, QN], F32, kind="ExternalOutput")
    with tile.TileContext(nc) as tc, ExitStack() as ctx:
        _build_body(ctx, tc, inp1.ap(), inp2.ap(), cst.ap(), out.ap())
    nc.compile()
    return nc


_NC = None


def _get_nc():
    global _NC
    if _NC is None:
        _NC = build_program()
    return _NC


def _build_consts(rpn_w):
    cst = np.zeros((128, CW), np.float32)
    # qmask: descriptor d = 15p + j belongs to query q = d // 25
    pp = np.arange(P)
    for j in range(QT):
        cst[pp, QM0 + QN * j + (15 * pp + j) // HW] = 1.0 / HW
    for j in range(ST):
        cst[pp, SM0 + NMAP * j + (5 * pp + j) // HW] = 1.0
        cst[pp, SM2 + NMAP * j + (5 * pp + j) // HW] = 1.0 / HW
    cst[:, WB0:WB0 + C] = np.asarray(rpn_w, np.float32).reshape(1, C)
    cst[np.arange(QN), ID0 + np.arange(QN)] = 1.0
    m = np.arange(NMAP)
    cst[m, SH0 + m // SHOT] = 1.0 / SHOT
    return cst.astype(ml_dtypes.bfloat16)


def shard_inputs(input1, input2, rpn_w, rpn_b=None):
    """Shard over episodes; [E, 1875, 640] -> [E, 125, 15*640] is a pure
    reshape (descriptor d = 15p + j, slots consecutive in DRAM)."""
    i1 = np.asarray(input1, np.float32).reshape(B, P, QT * C).astype(
        ml_dtypes.bfloat16)
    i2 = np.asarray(input2, np.float32).reshape(B, P, ST * C).astype(
        ml_dtypes.bfloat16)
    cst = _build_consts(rpn_w)
    in_maps = []
    for i in range(NCORES):
        in_maps.append({
            "input1": np.ascontiguousarray(i1[E * i:E * (i + 1)]),
            "input2": np.ascontiguousarray(i2[E * i:E * (i + 1)]),
            "consts": cst,
        })
    return in_maps


def _ensure_ntff_hook():
    """Install the NTFF profile hook (the image's antenv lacks axon_hooks)."""
    import types
    import antenv

    if "antenv.axon_hooks" not in sys.modules:
        mod = types.ModuleType("antenv.axon_hooks")
        mod._hook = None
        mod.set_axon_ntff_profile_hook = lambda h: setattr(mod, "_hook", h)
        mod.get_axon_ntff_profile_hook = lambda: mod._hook
        sys.modules["antenv.axon_hooks"] = mod
        antenv.axon_hooks = mod
    mod = sys.modules["antenv.axon_hooks"]
    if mod.get_axon_ntff_profile_hook() is None:
        from trn_agent_boot.trn_boot import _ntff_profile_via_ctypes
        hook = _ntff_profile_via_ctypes("/opt/axon/libaxon_pjrt.so")
        if hook is not None:
            mod.set_axon_ntff_profile_hook(hook)


def kernel(input1, input2, rpn_w, rpn_b=None, **run_kwargs):
    if run_kwargs.get("trace"):
        _ensure_ntff_hook()
    nc = _get_nc()
    in_maps = shard_inputs(input1, input2, rpn_w)
    res = run_bass_kernel_spmd(nc, in_maps, list(range(NCORES)), **run_kwargs)
    # sim comes back [E, way, qn]; un-transpose on the host
    out = np.concatenate(
        [np.transpose(r["sim"], (0, 2, 1)) for r in res.results], axis=0)
    if run_kwargs:
        kernel.last_results = res
    return out.astype(np.float32)


# revision 28
# speedup vs baseline: 1.3091x; 1.0020x over previous
"""MetaBaseline (retrieval_knn) Trainium2 kernel.

Computation (per episode b):
  q  = l2norm(input1[b])            # [75, 25, 640] over channel
  s  = l2norm(input2[b])            # [5, 5, 25, 640]
  att = softmax_hw(s @ rpn_w)       # rpn_b is softmax-invariant
  cg  = leaky(sum_hw(att * s))
  feat = mean_shot(mean_hw(s) + 5 * cg)
  sim[b] = mean_hw(q) @ feat.T      # [75, 5]

Sharding: data-parallel over episodes, 4 per core on 8 cores.

Design (v3): bulk data moves and streams as bf16 (PE: 1 cycle/col vs 2
for fp16/fp32r; rel-err budget 2e-2, measured ~3.5e-3). Input DMAs are
SWDGE (gpsimd) — HWDGE 2D descriptor generation caps at ~130GB/s while
SWDGE sustains ~200 — and are issued before any other gpsimd work so the
SDMA engines stream continuously from t=0; the full per-core shard
(~52KB/partition) is preloaded, no buffer recycling. Constants (masks /
identity / broadcast-w) are host-precomputed, one small sync-DMA.
1/sqrt(n2) runs on ACT as exp(-0.5*ln(x)) — square, ln and exp live in
one table set — replacing a 10-op DVE Newton per batch with 2 ACT ops.
Per-slot mask scaling (inv-norm / att weights folded into PE stationary
masks) is batched into a few broadcast-AP DVE multiplies. The support
stream computes the attention-sum and the mean in ONE PE pass
(stationary [125, 57]); feat is produced directly transposed via
fp.T @ shotm; sim is computed as [way, qn] and un-transposed on the
host. The per-episode tail (qm transpose + sim) is software-pipelined
one episode behind the main passes so no engine queue stalls on a
cross-engine round trip.
"""

import os
import sys
from contextlib import ExitStack

sys.path.insert(0, "/opt/trn_rl_repo")

import numpy as np
import ml_dtypes

import concourse.bass as bass
import concourse.tile as tile
from concourse import bacc, mybir
from concourse.bass_utils import run_bass_kernel_spmd

# Pin every activation to the natural_log_exp_and_others table set (it
# holds square, exp AND ln). The default chooser maps each function to
# its "home" set, which thrashes ACT_TABLE_LOADs (~1.3us each) between
# Square and Ln/Exp; one shared set means exactly one load. Indices of
# the other sets are preserved (emptied, not removed) so the emitted
# act_func_set_id still matches act_info.json.
import concourse.bacc as _bacc_mod
from concourse.hw_specs import get_activation_tables as _orig_act_tables

_ACT_SET = "natural_log_exp_and_others"


def _pinned_act_tables(arch):
    return {k: (v if k == _ACT_SET else set())
            for k, v in _orig_act_tables(arch).items()}


_bacc_mod.get_activation_tables = _pinned_act_tables

# The Tile scheduler orders each engine's queue with a cost model that
# assumes 360GB/s DMA. With 8 cores streaming concurrently a core
# really gets ~210GB/s, so the scheduler parks DMA-gated ops ahead of
# already-ready compute and the in-order engine queues stall on arrival
# (observed: a q-chunk Square scheduled between a rsqrt's Ln and Exp,
# blocking ACT ~6us). Feeding the scheduler the measured rate fixes the
# ordering. These class attrs are re-read at every schedule invocation.
import concourse.hw_specs as _hw_specs

_hw_specs.TRN2Spec.DMA_BUS_BYTES_PER_NS_PER_ENGINE = 210e9 / 16 / 1e9
_hw_specs.TRN2Spec.DMA_CYCLE = 1e9 / (210e9 / 128)

F32 = mybir.dt.float32
BF = mybir.dt.bfloat16
AX = mybir.AxisListType
OP = mybir.AluOpType
AF = mybir.ActivationFunctionType

# Problem constants (fixed by the problem statement).
B, QN, WAY, SHOT, HH, WW, C = 32, 75, 5, 5, 5, 5, 640
NCORES = 8
E = B // NCORES        # 4 episodes per core
HW = HH * WW           # 25 spatial positions
QD = QN * HW           # 1875 query descriptors / episode
SD = WAY * SHOT * HW   # 625 support descriptors / episode
P = 125                # descriptors per tile
QT = QD // P           # 15 query slots / episode (desc d = 15p + j)
ST = SD // P           # 5 support slots / episode (desc d = 5p + j)
NMAP = WAY * SHOT      # 25 support maps / episode
NCH = 3                # q DMA chunks (5 slots each)
SPC = QT // NCH        # slots per chunk
GAMMA = 5.0
SLOPE = 0.01
CH = C // 2            # 320-column halves (one PSUM bank each)
MB = 32                # mean-row base partition in the fused support psum
SW = MB + NMAP         # fused stationary width (57)
QNP = QN + 1           # padded transpose chunk stride (PSUM 4B align)

# constants tensor layout (free-axis offsets, bf16)
QM0 = 0                    # qmasks  [125, 15*75], value 1/25
SM0 = QM0 + QT * QN        # smasks  [125, 5*25],  value 1.0 (sums + att)
SM2 = SM0 + ST * NMAP      # smasks  [125, 5*25],  value 1/25 (hw-mean)
WB0 = SM2 + ST * NMAP      # w bcast [128, 640]
ID0 = WB0 + C              # identity [75, 75]
SH0 = ID0 + QN             # shotm   [25, 5], value 1/5
CW = SH0 + WAY             # = 2095

# engine split of the per-slot norm passes (True -> ACT)
S_ACT = (True, True, True, False, False)
Q_ACT = (True, True, True, True, False,
         True, True, True, False, False,
         True, True, False, False, False)


def _build_body(ctx: ExitStack, tc: "tile.TileContext", i1, i2, cst, out):
    nc = tc.nc

    cpool = ctx.enter_context(tc.tile_pool(name="consts", bufs=1))
    dpool = ctx.enter_context(tc.tile_pool(name="data", bufs=1))
    scr_pool = ctx.enter_context(tc.tile_pool(name="scratch", bufs=1))
    stats = ctx.enter_context(tc.tile_pool(name="stats", bufs=2))
    sel_pool = ctx.enter_context(tc.tile_pool(name="sel", bufs=2))
    sb_pool = ctx.enter_context(tc.tile_pool(name="sbwork", bufs=2))
    ps = ctx.enter_context(tc.tile_pool(name="ps", bufs=1, space="PSUM"))

    # ---- all input DMAs first (SWDGE; gpsimd queue head) ----
    # s_sl[e][j] is the [125, 640] slot view; episode 0's support tensor is
    # loaded as 5 per-slot DMAs so the very first compute tile lands ~10us
    # earlier (a big DMA's completion waits on all 125 descriptors).
    s_sl, q_t = [], []
    for e in range(E):
        if e == 0:
            sl = []
            for j in range(ST):
                t_ = dpool.tile([P, C], BF, name=f"s0_{j}", tag=f"s0_{j}")
                nc.gpsimd.dma_start(t_[:], i2[0, :, C * j:C * (j + 1)])
                sl.append(t_[:])
        else:
            st_ = dpool.tile([P, ST * C], BF, name=f"s_{e}", tag=f"s_{e}")
            nc.gpsimd.dma_start(st_[:], i2[e])
            sl = [st_[:, C * j:C * (j + 1)] for j in range(ST)]
        qc = []
        for c in range(NCH):
            qt_ = dpool.tile([P, SPC * C], BF, name=f"q_{e}_{c}",
                             tag=f"q_{e}_{c}")
            nc.gpsimd.dma_start(qt_[:], i1[e, :, SPC * C * c:SPC * C * (c + 1)])
            qc.append(qt_)
        s_sl.append(sl)
        q_t.append(qc)

    # ---- constants (host-precomputed, one sync DMA) ----
    consts = cpool.tile([128, CW], BF, name="consts")
    nc.sync.dma_start(consts[:], cst)
    smask = [consts[0:P, SM0 + NMAP * j:SM0 + NMAP * (j + 1)] for j in range(ST)]
    smask3 = consts[0:P, SM0:SM0 + ST * NMAP].rearrange(
        "p (j m) -> p j m", j=ST)
    smask3m = consts[0:P, SM2:SM2 + ST * NMAP].rearrange(
        "p (j m) -> p j m", j=ST)
    qmask3 = [consts[0:P, QM0 + SPC * QN * c:QM0 + SPC * QN * (c + 1)]
              .rearrange("p (j q) -> p j q", j=SPC) for c in range(NCH)]
    wbc = consts[0:P, WB0:WB0 + C]
    ident = consts[0:QN, ID0:ID0 + QN]
    shotm = consts[0:NMAP, SH0:SH0 + WAY]

    # fused support stationary [125, ST, 57] (cols 25-31 stay zero forever)
    st_all = cpool.tile([P, ST, SW], BF, name="st_all")
    nc.vector.memset(st_all[:, :, NMAP:MB], 0.0)

    def slot(big, j):
        return big[:, C * j:C * (j + 1)]

    def pe_tickle(anchor_ap):
        """Dummy 1-column LDWEIGHTS anchored to a freshly-written bf16
        tile. Executes in ~85ns as soon as the anchor is ready, keeping
        the PE HAM activity monitor from re-throttling the clock to
        1.2GHz during ACT/DVE-bound phases (idle windows >3.4us drop the
        PE to K=4/8). All real matmuls self-load weights, so clobbering
        the stationary register is safe."""
        nc.tensor.ldweights(anchor_ap)

    def rsqrt_act(dst, x, n, tag):
        """dst = 1/sqrt(x) on ACT: exp(-0.5*ln(x)); same table set as
        Square/Exp, so no ACT_TABLE_LOAD switches."""
        t = stats.tile([P, n], F32, name=f"rs_{tag}", tag=f"rs_{tag}")
        nc.scalar.activation(t[:], x, AF.Ln)
        nc.scalar.activation(dst, t[:], AF.Exp, scale=-0.5)

    def norm_pass(sl, acc_col, on_act):
        if on_act:
            scr = scr_pool.tile([P, C], BF, name="sq_a", tag="sq_a")
            nc.scalar.activation(scr[:], sl, AF.Square, accum_out=acc_col)
        else:
            scr = scr_pool.tile([P, C], BF, name="sq_v", tag="sq_v")
            nc.vector.scalar_tensor_tensor(
                out=scr[:], in0=sl, scalar=1.0, in1=sl,
                op0=OP.mult, op1=OP.mult, accum_out=acc_col)
        return scr

    # per-episode state carried into the pipelined tail
    qm_sb_t, ftT_t, tq_t, qmT_t = [None] * E, [None] * E, [None] * E, [None] * E

    def emit_tail_a(e):
        """PE transpose of qm (needs qm_sb[e]), on the prior episode's
        psum bank."""
        tq_ps = ps.tile([128, WAY * QNP], BF, name=f"tq_{e}", tag="tq")
        for cc in range(WAY):
            nc.tensor.transpose(tq_ps[:, QNP * cc:QNP * cc + QN],
                                qm_sb_t[e][:, 128 * cc:128 * (cc + 1)], ident)
        tq_t[e] = tq_ps
        qmT = sb_pool.tile([128, WAY * QNP], BF, name=f"qmT_{e}", tag="qmT")
        nc.scalar.copy(qmT[:], tq_ps[:])
        qmT_t[e] = qmT

    def emit_tail_b(e):
        sim_ps = ps.tile([WAY, QN], F32, name=f"sim_{e}", tag="sim")
        for cc in range(WAY):
            nc.tensor.matmul(sim_ps[:], ftT_t[e][:, WAY * cc:WAY * (cc + 1)],
                             qmT_t[e][:, QNP * cc:QNP * cc + QN],
                             start=(cc == 0), stop=(cc == WAY - 1))
        sim_sb = sb_pool.tile([WAY, QN], F32, name=f"sim_sb_{e}", tag="sim_sb")
        nc.vector.tensor_copy(sim_sb[:], sim_ps[:])
        nc.sync.dma_start(out[e], sim_sb[:])

    for e in range(E):
        ssl = s_sl[e]
        # ================= support side =================
        sn2 = stats.tile([P, ST], F32, name=f"sn2_{e}", tag="sn2")
        rr = stats.tile([P, ST], F32, name=f"rr_{e}", tag="rr")
        # DVE s-norms first so ACT's rsqrt isn't stuck behind the logits
        for j in range(ST):
            if not S_ACT[j]:
                norm_pass(ssl[j], sn2[:, j:j + 1], False)
        for j in range(ST):
            if S_ACT[j]:
                norm_pass(ssl[j], sn2[:, j:j + 1], True)
        for j in range(ST):
            scr2 = scr_pool.tile([P, C], BF, name="s_tt", tag="s_tt")
            nc.vector.scalar_tensor_tensor(
                out=scr2[:], in0=ssl[j], scalar=1.0, in1=wbc,
                op0=OP.mult, op1=OP.mult, accum_out=rr[:, j:j + 1])
        sinv = stats.tile([P, ST], BF, name=f"sinv_{e}", tag="sinv")
        rsqrt_act(sinv[:], sn2[:], ST, f"s{e % 2}")
        # softmax over hw within each map (logits tiny: no max-shift)
        lg = stats.tile([P, ST], F32, name=f"lg_{e}", tag="lg")
        nc.vector.tensor_mul(lg[:], rr[:], sinv[:])
        el = stats.tile([P, ST], BF, name=f"el_{e}", tag="el")
        nc.scalar.activation(el[:], lg[:], AF.Exp)
        # per-map sums of exp -> softmax reciprocal
        sums = ps.tile([NMAP, 1], F32, name=f"sums_{e}", tag="sums")
        for j in range(ST):
            nc.tensor.matmul(sums[:], smask[j], el[:, j:j + 1],
                             start=(j == 0), stop=(j == ST - 1))
        rec = stats.tile([NMAP, 1], F32, name=f"rec_{e}", tag="rec")
        nc.vector.reciprocal(rec[:], sums[:])
        # unnormalized att weights (softmax recip applied at cg evacuation)
        uw = stats.tile([P, ST], BF, name=f"uw_{e}", tag="uw")
        nc.vector.tensor_mul(uw[:], el[:], sinv[:])
        # batched stationary builds (pad cols stay zero)
        nc.vector.tensor_mul(st_all[:, :, 0:NMAP], smask3,
                             uw[:].broadcast_to((P, ST, NMAP)))
        nc.vector.tensor_mul(st_all[:, :, MB:SW], smask3m,
                             sinv[:].broadcast_to((P, ST, NMAP)))
        cg_ps = [ps.tile([SW, CH], F32, name=f"cg{h}_{e}", tag=f"cg{h}")
                 for h in range(2)]
        for j in range(ST):
            for h in range(2):
                nc.tensor.matmul(cg_ps[h][:], st_all[:, j, :],
                                 ssl[j][:, CH * h:CH * (h + 1)],
                                 start=(j == 0), stop=(j == ST - 1))
        # evacuate: cg rows 0-24 (x softmax recip), mean rows MB..MB+24
        cg_sb = sb_pool.tile([NMAP, C], F32, name=f"cg_sb_{e}", tag="cg_sb")
        for h in range(2):
            nc.vector.tensor_scalar_mul(cg_sb[:, CH * h:CH * (h + 1)],
                                        cg_ps[h][0:NMAP, :], rec[:, 0:1])
        lk = sb_pool.tile([NMAP, C], F32, name=f"lk_{e}", tag="lk")
        nc.vector.scalar_tensor_tensor(
            out=lk[:], in0=cg_sb[:], scalar=SLOPE, in1=cg_sb[:],
            op0=OP.mult, op1=OP.max)
        fp = sb_pool.tile([NMAP, C], BF, name=f"fp_{e}", tag="fp")
        for h in range(2):
            nc.vector.scalar_tensor_tensor(
                out=fp[:, CH * h:CH * (h + 1)], in0=lk[:, CH * h:CH * (h + 1)],
                scalar=GAMMA, in1=cg_ps[h][MB:MB + NMAP, :],
                op0=OP.mult, op1=OP.add)
        # featT[c, w] directly: fp.T @ shotm, chunked over c
        ftT_ps = ps.tile([128, WAY * WAY], F32, name=f"ftT_{e}", tag="ftT")
        for cc in range(WAY):
            nc.tensor.matmul(ftT_ps[:, WAY * cc:WAY * (cc + 1)],
                             fp[:, 128 * cc:128 * (cc + 1)], shotm)
        ftT = sb_pool.tile([128, WAY * WAY], BF, name=f"ftTs_{e}", tag="ftTs")
        nc.vector.tensor_copy(ftT[:], ftT_ps[:])
        ftT_t[e] = ftT

        # ================= query side =================
        qn2 = stats.tile([P, QT], F32, name=f"qn2_{e}", tag="qn2")
        qinv = stats.tile([P, QT], BF, name=f"qinv_{e}", tag="qinv")
        qm = [ps.tile([QN, CH], F32, name=f"qm{h}_{e}", tag=f"qm{h}")
              for h in range(2)]
        for c in range(NCH):
            for jj in range(SPC):
                j = SPC * c + jj
                norm_pass(slot(q_t[e][c], jj), qn2[:, j:j + 1], Q_ACT[j])
            lo = SPC * c
            rsqrt_act(qinv[:, lo:lo + SPC], qn2[:, lo:lo + SPC], SPC,
                      f"q{c}_{e % 2}")
            sel = sel_pool.tile([P, SPC, QN], BF, name=f"sel{c}",
                                tag=f"sel{c}")
            nc.vector.tensor_mul(sel[:], qmask3[c],
                                 qinv[:, lo:lo + SPC]
                                 .broadcast_to((P, SPC, QN)))
            for jj in range(SPC):
                j = lo + jj
                for h in range(2):
                    nc.tensor.matmul(
                        qm[h][:], sel[:, jj, :],
                        slot(q_t[e][c], jj)[:, CH * h:CH * (h + 1)],
                        start=(j == 0), stop=(j == QT - 1))
            # spread the previous episode's tail into the q-norm gaps
            if e > 0:
                if c == 0:
                    emit_tail_a(e - 1)
                elif c == 1:
                    emit_tail_b(e - 1)
        qm_sb = sb_pool.tile([QN, C], BF, name=f"qm_sb_{e}", tag="qm_sb")
        for h in range(2):
            nc.scalar.copy(qm_sb[:, CH * h:CH * (h + 1)], qm[h][:])
        qm_sb_t[e] = qm_sb
    emit_tail_a(E - 1)
    emit_tail_b(E - 1)


def build_program():
    nc = bacc.Bacc("TRN2", target_bir_lowering=False, debug=False,
                   num_devices=NCORES)
    inp1 = nc.dram_tensor("input1", [E, P, QT * C], BF, kind="ExternalInput")
    inp2 = nc.dram_tensor("input2", [E, P, ST * C], BF, kind="ExternalInput")
    cst = nc.dram_tensor("consts", [128, CW], BF, kind="ExternalInput")
    out = nc.dram_tensor("sim", [E, WAY, QN], F32, kind="ExternalOutput")
    with tile.TileContext(nc) as tc, ExitStack() as ctx:
        _build_body(ctx, tc, inp1.ap(), inp2.ap(), cst.ap(), out.ap())
    nc.compile()
    return nc


_NC = None


def _get_nc():
    global _NC
    if _NC is None:
        _NC = build_program()
    return _NC


def _build_consts(rpn_w):
    cst = np.zeros((128, CW), np.float32)
    # qmask: descriptor d = 15p + j belongs to query q = d // 25
    pp = np.arange(P)
    for j in range(QT):
        cst[pp, QM0 + QN * j + (15 * pp + j) // HW] = 1.0 / HW
    for j in range(ST):
        cst[pp, SM0 + NMAP * j + (5 * pp + j) // HW] = 1.0
        cst[pp, SM2 + NMAP * j + (5 * pp + j) // HW] = 1.0 / HW
    cst[:, WB0:WB0 + C] = np.asarray(rpn_w, np.float32).reshape(1, C)
    cst[np.arange(QN), ID0 + np.arange(QN)] = 1.0
    m = np.arange(NMAP)
    cst[m, SH0 + m // SHOT] = 1.0 / SHOT
    return cst.astype(ml_dtypes.bfloat16)


def shard_inputs(input1, input2, rpn_w, rpn_b=None):
    """Shard over episodes; [E, 1875, 640] -> [E, 125, 15*640] is a pure
    reshape (descriptor d = 15p + j, slots consecutive in DRAM)."""
    i1 = np.asarray(input1, np.float32).reshape(B, P, QT * C).astype(
        ml_dtypes.bfloat16)
    i2 = np.asarray(input2, np.float32).reshape(B, P, ST * C).astype(
        ml_dtypes.bfloat16)
    cst = _build_consts(rpn_w)
    in_maps = []
    for i in range(NCORES):
        in_maps.append({
            "input1": np.ascontiguousarray(i1[E * i:E * (i + 1)]),
            "input2": np.ascontiguousarray(i2[E * i:E * (i + 1)]),
            "consts": cst,
        })
    return in_maps


def _ensure_ntff_hook():
    """Install the NTFF profile hook (the image's antenv lacks axon_hooks)."""
    import types
    import antenv

    if "antenv.axon_hooks" not in sys.modules:
        mod = types.ModuleType("antenv.axon_hooks")
        mod._hook = None
        mod.set_axon_ntff_profile_hook = lambda h: setattr(mod, "_hook", h)
        mod.get_axon_ntff_profile_hook = lambda: mod._hook
        sys.modules["antenv.axon_hooks"] = mod
        antenv.axon_hooks = mod
    mod = sys.modules["antenv.axon_hooks"]
    if mod.get_axon_ntff_profile_hook() is None:
        from trn_agent_boot.trn_boot import _ntff_profile_via_ctypes
        hook = _ntff_profile_via_ctypes("/opt/axon/libaxon_pjrt.so")
        if hook is not None:
            mod.set_axon_ntff_profile_hook(hook)


def kernel(input1, input2, rpn_w, rpn_b=None, **run_kwargs):
    if run_kwargs.get("trace"):
        _ensure_ntff_hook()
    nc = _get_nc()
    in_maps = shard_inputs(input1, input2, rpn_w)
    res = run_bass_kernel_spmd(nc, in_maps, list(range(NCORES)), **run_kwargs)
    # sim comes back [E, way, qn]; un-transpose on the host
    out = np.concatenate(
        [np.transpose(r["sim"], (0, 2, 1)) for r in res.results], axis=0)
    if run_kwargs:
        kernel.last_results = res
    return out.astype(np.float32)


# revision 33
# speedup vs baseline: 1.3166x; 1.0057x over previous
"""MetaBaseline (retrieval_knn) Trainium2 kernel.

Computation (per episode b):
  q  = l2norm(input1[b])            # [75, 25, 640] over channel
  s  = l2norm(input2[b])            # [5, 5, 25, 640]
  att = softmax_hw(s @ rpn_w)       # rpn_b is softmax-invariant
  cg  = leaky(sum_hw(att * s))
  feat = mean_shot(mean_hw(s) + 5 * cg)
  sim[b] = mean_hw(q) @ feat.T      # [75, 5]

Sharding: data-parallel over episodes, 4 per core on 8 cores.

Design (v3): bulk data moves and streams as bf16 (PE: 1 cycle/col vs 2
for fp16/fp32r; rel-err budget 2e-2, measured ~3.5e-3). Input DMAs are
SWDGE (gpsimd) — HWDGE 2D descriptor generation caps at ~130GB/s while
SWDGE sustains ~200 — and are issued before any other gpsimd work so the
SDMA engines stream continuously from t=0; the full per-core shard
(~52KB/partition) is preloaded, no buffer recycling. Constants (masks /
identity / broadcast-w) are host-precomputed, one small sync-DMA.
1/sqrt(n2) runs on ACT as exp(-0.5*ln(x)) — square, ln and exp live in
one table set — replacing a 10-op DVE Newton per batch with 2 ACT ops.
Per-slot mask scaling (inv-norm / att weights folded into PE stationary
masks) is batched into a few broadcast-AP DVE multiplies. The support
stream computes the attention-sum and the mean in ONE PE pass
(stationary [125, 57]); feat is produced directly transposed via
fp.T @ shotm; sim is computed as [way, qn] and un-transposed on the
host. The per-episode tail (qm transpose + sim) is software-pipelined
one episode behind the main passes so no engine queue stalls on a
cross-engine round trip.
"""

import os
import sys
from contextlib import ExitStack

sys.path.insert(0, "/opt/trn_rl_repo")

import numpy as np
import ml_dtypes

import concourse.bass as bass
import concourse.tile as tile
from concourse import bacc, mybir
from concourse.bass_utils import run_bass_kernel_spmd

# Pin every activation to the natural_log_exp_and_others table set (it
# holds square, exp AND ln). The default chooser maps each function to
# its "home" set, which thrashes ACT_TABLE_LOADs (~1.3us each) between
# Square and Ln/Exp; one shared set means exactly one load. Indices of
# the other sets are preserved (emptied, not removed) so the emitted
# act_func_set_id still matches act_info.json.
import concourse.bacc as _bacc_mod
from concourse.hw_specs import get_activation_tables as _orig_act_tables

_ACT_SET = "natural_log_exp_and_others"


def _pinned_act_tables(arch):
    return {k: (v if k == _ACT_SET else set())
            for k, v in _orig_act_tables(arch).items()}


_bacc_mod.get_activation_tables = _pinned_act_tables

# The Tile scheduler orders each engine's queue with a cost model that
# assumes 360GB/s DMA. With 8 cores streaming concurrently a core
# really gets ~210GB/s, so the scheduler parks DMA-gated ops ahead of
# already-ready compute and the in-order engine queues stall on arrival
# (observed: a q-chunk Square scheduled between a rsqrt's Ln and Exp,
# blocking ACT ~6us). Feeding the scheduler the measured rate fixes the
# ordering. These class attrs are re-read at every schedule invocation.
import concourse.hw_specs as _hw_specs

_hw_specs.TRN2Spec.DMA_BUS_BYTES_PER_NS_PER_ENGINE = 210e9 / 16 / 1e9
_hw_specs.TRN2Spec.DMA_CYCLE = 1e9 / (210e9 / 128)

F32 = mybir.dt.float32
BF = mybir.dt.bfloat16
AX = mybir.AxisListType
OP = mybir.AluOpType
AF = mybir.ActivationFunctionType

# Problem constants (fixed by the problem statement).
B, QN, WAY, SHOT, HH, WW, C = 32, 75, 5, 5, 5, 5, 640
NCORES = 8
E = B // NCORES        # 4 episodes per core
HW = HH * WW           # 25 spatial positions
QD = QN * HW           # 1875 query descriptors / episode
SD = WAY * SHOT * HW   # 625 support descriptors / episode
P = 125                # descriptors per tile
QT = QD // P           # 15 query slots / episode (desc d = 15p + j)
ST = SD // P           # 5 support slots / episode (desc d = 5p + j)
NMAP = WAY * SHOT      # 25 support maps / episode
NCH = 3                # q DMA chunks (5 slots each)
SPC = QT // NCH        # slots per chunk
GAMMA = 5.0
SLOPE = 0.01
CH = C // 2            # 320-column halves (one PSUM bank each)
MB = 32                # mean-row base partition in the fused support psum
SW = MB + NMAP         # fused stationary width (57)
QNP = QN + 1           # padded transpose chunk stride (PSUM 4B align)

# constants tensor layout (free-axis offsets, bf16)
QM0 = 0                    # qmasks  [125, 15*75], value 1/25
SM0 = QM0 + QT * QN        # smasks  [125, 5*25],  value 1.0 (sums + att)
SM2 = SM0 + ST * NMAP      # smasks  [125, 5*25],  value 1/25 (hw-mean)
WB0 = SM2 + ST * NMAP      # w bcast [128, 640]
ID0 = WB0 + C              # identity [75, 75]
SH0 = ID0 + QN             # shotm   [25, 5], value 1/5
CW = SH0 + WAY             # = 2095

# engine split of the per-slot norm passes (True -> ACT)
S_ACT = (True, True, True, False, False)
Q_ACT = (True, True, True, True, False,
         True, True, True, False, False,
         True, True, False, False, False)


def _build_body(ctx: ExitStack, tc: "tile.TileContext", i1, i2, cst, out):
    nc = tc.nc

    cpool = ctx.enter_context(tc.tile_pool(name="consts", bufs=1))
    dpool = ctx.enter_context(tc.tile_pool(name="data", bufs=1))
    scr_pool = ctx.enter_context(tc.tile_pool(name="scratch", bufs=1))
    stats = ctx.enter_context(tc.tile_pool(name="stats", bufs=2))
    sel_pool = ctx.enter_context(tc.tile_pool(name="sel", bufs=2))
    sb_pool = ctx.enter_context(tc.tile_pool(name="sbwork", bufs=2))
    ps = ctx.enter_context(tc.tile_pool(name="ps", bufs=1, space="PSUM"))

    # ---- all input DMAs first (SWDGE; gpsimd queue head) ----
    # s_sl[e][j] is the [125, 640] slot view; episode 0's support tensor is
    # loaded as 5 per-slot DMAs so the very first compute tile lands ~10us
    # earlier (a big DMA's completion waits on all 125 descriptors).
    s_sl, q_t = [], []
    for e in range(E):
        if e == 0:
            sl = []
            for j in range(ST):
                t_ = dpool.tile([P, C], BF, name=f"s0_{j}", tag=f"s0_{j}")
                nc.gpsimd.dma_start(t_[:], i2[0, :, C * j:C * (j + 1)])
                sl.append(t_[:])
        else:
            st_ = dpool.tile([P, ST * C], BF, name=f"s_{e}", tag=f"s_{e}")
            nc.gpsimd.dma_start(st_[:], i2[e])
            sl = [st_[:, C * j:C * (j + 1)] for j in range(ST)]
        qc = []
        for c in range(NCH):
            if e == 0 and c == 0:
                # episode 0 chunk 0 also lands per-slot so the q-norm
                # pipeline starts ~7us earlier
                qsl = []
                for jj in range(SPC):
                    t_ = dpool.tile([P, C], BF, name=f"q00_{jj}",
                                    tag=f"q00_{jj}")
                    nc.gpsimd.dma_start(t_[:], i1[0, :, C * jj:C * (jj + 1)])
                    qsl.append(t_[:])
                qc.append(qsl)
            else:
                qt_ = dpool.tile([P, SPC * C], BF, name=f"q_{e}_{c}",
                                 tag=f"q_{e}_{c}")
                nc.gpsimd.dma_start(qt_[:],
                                    i1[e, :, SPC * C * c:SPC * C * (c + 1)])
                qc.append([qt_[:, C * jj:C * (jj + 1)] for jj in range(SPC)])
        s_sl.append(sl)
        q_t.append(qc)

    # ---- constants (host-precomputed, one sync DMA) ----
    consts = cpool.tile([128, CW], BF, name="consts")
    nc.sync.dma_start(consts[:], cst)
    smask = [consts[0:P, SM0 + NMAP * j:SM0 + NMAP * (j + 1)] for j in range(ST)]
    smask3 = consts[0:P, SM0:SM0 + ST * NMAP].rearrange(
        "p (j m) -> p j m", j=ST)
    smask3m = consts[0:P, SM2:SM2 + ST * NMAP].rearrange(
        "p (j m) -> p j m", j=ST)
    qmask3 = [consts[0:P, QM0 + SPC * QN * c:QM0 + SPC * QN * (c + 1)]
              .rearrange("p (j q) -> p j q", j=SPC) for c in range(NCH)]
    wbc = consts[0:P, WB0:WB0 + C]
    ident = consts[0:QN, ID0:ID0 + QN]
    shotm = consts[0:NMAP, SH0:SH0 + WAY]

    # fused support stationary [125, ST, 57] (cols 25-31 stay zero forever)
    st_all = cpool.tile([P, ST, SW], BF, name="st_all")
    nc.vector.memset(st_all[:, :, NMAP:MB], 0.0)

    def slot(big, j):
        return big[:, C * j:C * (j + 1)]

    def pe_tickle(anchor_ap):
        """Dummy 1-column LDWEIGHTS anchored to a freshly-written bf16
        tile. Executes in ~85ns as soon as the anchor is ready, keeping
        the PE HAM activity monitor from re-throttling the clock to
        1.2GHz during ACT/DVE-bound phases (idle windows >3.4us drop the
        PE to K=4/8). All real matmuls self-load weights, so clobbering
        the stationary register is safe."""
        nc.tensor.ldweights(anchor_ap)

    def rsqrt_act(dst, x, n, tag):
        """dst = 1/sqrt(x) on ACT: exp(-0.5*ln(x)); same table set as
        Square/Exp, so no ACT_TABLE_LOAD switches."""
        t = stats.tile([P, n], F32, name=f"rs_{tag}", tag=f"rs_{tag}")
        nc.scalar.activation(t[:], x, AF.Ln)
        nc.scalar.activation(dst, t[:], AF.Exp, scale=-0.5)

    def norm_pass(sl, acc_col, on_act):
        if on_act:
            scr = scr_pool.tile([P, C], BF, name="sq_a", tag="sq_a")
            nc.scalar.activation(scr[:], sl, AF.Square, accum_out=acc_col)
        else:
            scr = scr_pool.tile([P, C], BF, name="sq_v", tag="sq_v")
            nc.vector.scalar_tensor_tensor(
                out=scr[:], in0=sl, scalar=1.0, in1=sl,
                op0=OP.mult, op1=OP.mult, accum_out=acc_col)
        return scr

    # per-episode state carried into the pipelined tail
    qm_sb_t, ftT_t, tq_t, qmT_t = [None] * E, [None] * E, [None] * E, [None] * E

    def emit_tail_a(e):
        """PE transpose of qm (needs qm_sb[e]), on the prior episode's
        psum bank."""
        tq_ps = ps.tile([128, WAY * QNP], BF, name=f"tq_{e}", tag="tq")
        for cc in range(WAY):
            nc.tensor.transpose(tq_ps[:, QNP * cc:QNP * cc + QN],
                                qm_sb_t[e][:, 128 * cc:128 * (cc + 1)], ident)
        tq_t[e] = tq_ps
        qmT = sb_pool.tile([128, WAY * QNP], BF, name=f"qmT_{e}", tag="qmT")
        nc.scalar.copy(qmT[:], tq_ps[:])
        qmT_t[e] = qmT

    def emit_tail_b(e):
        sim_ps = ps.tile([WAY, QN], F32, name=f"sim_{e}", tag="sim")
        for cc in range(WAY):
            nc.tensor.matmul(sim_ps[:], ftT_t[e][:, WAY * cc:WAY * (cc + 1)],
                             qmT_t[e][:, QNP * cc:QNP * cc + QN],
                             start=(cc == 0), stop=(cc == WAY - 1))
        sim_sb = sb_pool.tile([WAY, QN], F32, name=f"sim_sb_{e}", tag="sim_sb")
        nc.vector.tensor_copy(sim_sb[:], sim_ps[:])
        nc.sync.dma_start(out[e], sim_sb[:])

    for e in range(E):
        ssl = s_sl[e]
        # ================= support side =================
        sn2 = stats.tile([P, ST], F32, name=f"sn2_{e}", tag="sn2")
        rr = stats.tile([P, ST], F32, name=f"rr_{e}", tag="rr")
        # DVE s-norms first so ACT's rsqrt isn't stuck behind the logits
        for j in range(ST):
            if not S_ACT[j]:
                norm_pass(ssl[j], sn2[:, j:j + 1], False)
        for j in range(ST):
            if S_ACT[j]:
                norm_pass(ssl[j], sn2[:, j:j + 1], True)
        for j in range(ST):
            scr2 = scr_pool.tile([P, C], BF, name="s_tt", tag="s_tt")
            nc.vector.scalar_tensor_tensor(
                out=scr2[:], in0=ssl[j], scalar=1.0, in1=wbc,
                op0=OP.mult, op1=OP.mult, accum_out=rr[:, j:j + 1])
        sinv = stats.tile([P, ST], BF, name=f"sinv_{e}", tag="sinv")
        rsqrt_act(sinv[:], sn2[:], ST, f"s{e % 2}")
        # softmax over hw within each map (logits tiny: no max-shift)
        lg = stats.tile([P, ST], F32, name=f"lg_{e}", tag="lg")
        nc.vector.tensor_mul(lg[:], rr[:], sinv[:])
        el = stats.tile([P, ST], BF, name=f"el_{e}", tag="el")
        nc.scalar.activation(el[:], lg[:], AF.Exp)
        # per-map sums of exp -> softmax reciprocal
        sums = ps.tile([NMAP, 1], F32, name=f"sums_{e}", tag="sums")
        for j in range(ST):
            nc.tensor.matmul(sums[:], smask[j], el[:, j:j + 1],
                             start=(j == 0), stop=(j == ST - 1))
        rec = stats.tile([NMAP, 1], F32, name=f"rec_{e}", tag="rec")
        nc.vector.reciprocal(rec[:], sums[:])
        # unnormalized att weights (softmax recip applied at cg evacuation)
        uw = stats.tile([P, ST], BF, name=f"uw_{e}", tag="uw")
        nc.vector.tensor_mul(uw[:], el[:], sinv[:])
        # batched stationary builds (pad cols stay zero)
        nc.vector.tensor_mul(st_all[:, :, 0:NMAP], smask3,
                             uw[:].broadcast_to((P, ST, NMAP)))
        nc.vector.tensor_mul(st_all[:, :, MB:SW], smask3m,
                             sinv[:].broadcast_to((P, ST, NMAP)))
        cg_ps = [ps.tile([SW, CH], F32, name=f"cg{h}_{e}", tag=f"cg{h}")
                 for h in range(2)]
        for j in range(ST):
            for h in range(2):
                nc.tensor.matmul(cg_ps[h][:], st_all[:, j, :],
                                 ssl[j][:, CH * h:CH * (h + 1)],
                                 start=(j == 0), stop=(j == ST - 1))
        # evacuate: cg rows 0-24 (x softmax recip), mean rows MB..MB+24
        cg_sb = sb_pool.tile([NMAP, C], F32, name=f"cg_sb_{e}", tag="cg_sb")
        for h in range(2):
            nc.vector.tensor_scalar_mul(cg_sb[:, CH * h:CH * (h + 1)],
                                        cg_ps[h][0:NMAP, :], rec[:, 0:1])
        lk = sb_pool.tile([NMAP, C], F32, name=f"lk_{e}", tag="lk")
        nc.vector.scalar_tensor_tensor(
            out=lk[:], in0=cg_sb[:], scalar=SLOPE, in1=cg_sb[:],
            op0=OP.mult, op1=OP.max)
        fp = sb_pool.tile([NMAP, C], BF, name=f"fp_{e}", tag="fp")
        for h in range(2):
            nc.vector.scalar_tensor_tensor(
                out=fp[:, CH * h:CH * (h + 1)], in0=lk[:, CH * h:CH * (h + 1)],
                scalar=GAMMA, in1=cg_ps[h][MB:MB + NMAP, :],
                op0=OP.mult, op1=OP.add)
        # featT[c, w] directly: fp.T @ shotm, chunked over c
        ftT_ps = ps.tile([128, WAY * WAY], F32, name=f"ftT_{e}", tag="ftT")
        for cc in range(WAY):
            nc.tensor.matmul(ftT_ps[:, WAY * cc:WAY * (cc + 1)],
                             fp[:, 128 * cc:128 * (cc + 1)], shotm)
        ftT = sb_pool.tile([128, WAY * WAY], BF, name=f"ftTs_{e}", tag="ftTs")
        nc.vector.tensor_copy(ftT[:], ftT_ps[:])
        ftT_t[e] = ftT

        # ================= query side =================
        qn2 = stats.tile([P, QT], F32, name=f"qn2_{e}", tag="qn2")
        qinv = stats.tile([P, QT], BF, name=f"qinv_{e}", tag="qinv")
        qm = [ps.tile([QN, CH], F32, name=f"qm{h}_{e}", tag=f"qm{h}")
              for h in range(2)]
        for c in range(NCH):
            for jj in range(SPC):
                j = SPC * c + jj
                norm_pass(q_t[e][c][jj], qn2[:, j:j + 1], Q_ACT[j])
            lo = SPC * c
            rsqrt_act(qinv[:, lo:lo + SPC], qn2[:, lo:lo + SPC], SPC,
                      f"q{c}_{e % 2}")
            sel = sel_pool.tile([P, SPC, QN], BF, name=f"sel{c}",
                                tag=f"sel{c}")
            nc.vector.tensor_mul(sel[:], qmask3[c],
                                 qinv[:, lo:lo + SPC]
                                 .broadcast_to((P, SPC, QN)))
            for jj in range(SPC):
                j = lo + jj
                for h in range(2):
                    nc.tensor.matmul(
                        qm[h][:], sel[:, jj, :],
                        q_t[e][c][jj][:, CH * h:CH * (h + 1)],
                        start=(j == 0), stop=(j == QT - 1))
            # spread the previous episode's tail into the q-norm gaps
            if e > 0:
                if c == 0:
                    emit_tail_a(e - 1)
                elif c == 1:
                    emit_tail_b(e - 1)
        qm_sb = sb_pool.tile([QN, C], BF, name=f"qm_sb_{e}", tag="qm_sb")
        for h in range(2):
            nc.scalar.copy(qm_sb[:, CH * h:CH * (h + 1)], qm[h][:])
        qm_sb_t[e] = qm_sb
    emit_tail_a(E - 1)
    emit_tail_b(E - 1)


def build_program():
    nc = bacc.Bacc("TRN2", target_bir_lowering=False, debug=False,
                   num_devices=NCORES)
    inp1 = nc.dram_tensor("input1", [E, P, QT * C], BF, kind="ExternalInput")
    inp2 = nc.dram_tensor("input2", [E, P, ST * C], BF, kind="ExternalInput")
    cst = nc.dram_tensor("consts", [128, CW], BF, kind="ExternalInput")
    out = nc.dram_tensor("sim", [E, WAY, QN], F32, kind="ExternalOutput")
    with tile.TileContext(nc) as tc, ExitStack() as ctx:
        _build_body(ctx, tc, inp1.ap(), inp2.ap(), cst.ap(), out.ap())
    nc.compile()
    return nc


_NC = None


def _get_nc():
    global _NC
    if _NC is None:
        _NC = build_program()
    return _NC


def _build_consts(rpn_w):
    cst = np.zeros((128, CW), np.float32)
    # qmask: descriptor d = 15p + j belongs to query q = d // 25
    pp = np.arange(P)
    for j in range(QT):
        cst[pp, QM0 + QN * j + (15 * pp + j) // HW] = 1.0 / HW
    for j in range(ST):
        cst[pp, SM0 + NMAP * j + (5 * pp + j) // HW] = 1.0
        cst[pp, SM2 + NMAP * j + (5 * pp + j) // HW] = 1.0 / HW
    cst[:, WB0:WB0 + C] = np.asarray(rpn_w, np.float32).reshape(1, C)
    cst[np.arange(QN), ID0 + np.arange(QN)] = 1.0
    m = np.arange(NMAP)
    cst[m, SH0 + m // SHOT] = 1.0 / SHOT
    return cst.astype(ml_dtypes.bfloat16)


def shard_inputs(input1, input2, rpn_w, rpn_b=None):
    """Shard over episodes; [E, 1875, 640] -> [E, 125, 15*640] is a pure
    reshape (descriptor d = 15p + j, slots consecutive in DRAM)."""
    i1 = np.asarray(input1, np.float32).reshape(B, P, QT * C).astype(
        ml_dtypes.bfloat16)
    i2 = np.asarray(input2, np.float32).reshape(B, P, ST * C).astype(
        ml_dtypes.bfloat16)
    cst = _build_consts(rpn_w)
    in_maps = []
    for i in range(NCORES):
        in_maps.append({
            "input1": np.ascontiguousarray(i1[E * i:E * (i + 1)]),
            "input2": np.ascontiguousarray(i2[E * i:E * (i + 1)]),
            "consts": cst,
        })
    return in_maps


def _ensure_ntff_hook():
    """Install the NTFF profile hook (the image's antenv lacks axon_hooks)."""
    import types
    import antenv

    if "antenv.axon_hooks" not in sys.modules:
        mod = types.ModuleType("antenv.axon_hooks")
        mod._hook = None
        mod.set_axon_ntff_profile_hook = lambda h: setattr(mod, "_hook", h)
        mod.get_axon_ntff_profile_hook = lambda: mod._hook
        sys.modules["antenv.axon_hooks"] = mod
        antenv.axon_hooks = mod
    mod = sys.modules["antenv.axon_hooks"]
    if mod.get_axon_ntff_profile_hook() is None:
        from trn_agent_boot.trn_boot import _ntff_profile_via_ctypes
        hook = _ntff_profile_via_ctypes("/opt/axon/libaxon_pjrt.so")
        if hook is not None:
            mod.set_axon_ntff_profile_hook(hook)


def kernel(input1, input2, rpn_w, rpn_b=None, **run_kwargs):
    if run_kwargs.get("trace"):
        _ensure_ntff_hook()
    nc = _get_nc()
    in_maps = shard_inputs(input1, input2, rpn_w)
    res = run_bass_kernel_spmd(nc, in_maps, list(range(NCORES)), **run_kwargs)
    # sim comes back [E, way, qn]; un-transpose on the host
    out = np.concatenate(
        [np.transpose(r["sim"], (0, 2, 1)) for r in res.results], axis=0)
    if run_kwargs:
        kernel.last_results = res
    return out.astype(np.float32)


# revision 34
# speedup vs baseline: 1.3194x; 1.0021x over previous
"""MetaBaseline (retrieval_knn) Trainium2 kernel.

Computation (per episode b):
  q  = l2norm(input1[b])            # [75, 25, 640] over channel
  s  = l2norm(input2[b])            # [5, 5, 25, 640]
  att = softmax_hw(s @ rpn_w)       # rpn_b is softmax-invariant
  cg  = leaky(sum_hw(att * s))
  feat = mean_shot(mean_hw(s) + 5 * cg)
  sim[b] = mean_hw(q) @ feat.T      # [75, 5]

Sharding: data-parallel over episodes, 4 per core on 8 cores.

Design (v3): bulk data moves and streams as bf16 (PE: 1 cycle/col vs 2
for fp16/fp32r; rel-err budget 2e-2, measured ~3.5e-3). Input DMAs are
SWDGE (gpsimd) — HWDGE 2D descriptor generation caps at ~130GB/s while
SWDGE sustains ~200 — and are issued before any other gpsimd work so the
SDMA engines stream continuously from t=0; the full per-core shard
(~52KB/partition) is preloaded, no buffer recycling. Constants (masks /
identity / broadcast-w) are host-precomputed, one small sync-DMA.
1/sqrt(n2) runs on ACT as exp(-0.5*ln(x)) — square, ln and exp live in
one table set — replacing a 10-op DVE Newton per batch with 2 ACT ops.
Per-slot mask scaling (inv-norm / att weights folded into PE stationary
masks) is batched into a few broadcast-AP DVE multiplies. The support
stream computes the attention-sum and the mean in ONE PE pass
(stationary [125, 57]); feat is produced directly transposed via
fp.T @ shotm; sim is computed as [way, qn] and un-transposed on the
host. The per-episode tail (qm transpose + sim) is software-pipelined
one episode behind the main passes so no engine queue stalls on a
cross-engine round trip.
"""

import os
import sys
from contextlib import ExitStack

sys.path.insert(0, "/opt/trn_rl_repo")

import numpy as np
import ml_dtypes

import concourse.bass as bass
import concourse.tile as tile
from concourse import bacc, mybir
from concourse.bass_utils import run_bass_kernel_spmd

# Pin every activation to the natural_log_exp_and_others table set (it
# holds square, exp AND ln). The default chooser maps each function to
# its "home" set, which thrashes ACT_TABLE_LOADs (~1.3us each) between
# Square and Ln/Exp; one shared set means exactly one load. Indices of
# the other sets are preserved (emptied, not removed) so the emitted
# act_func_set_id still matches act_info.json.
import concourse.bacc as _bacc_mod
from concourse.hw_specs import get_activation_tables as _orig_act_tables

_ACT_SET = "natural_log_exp_and_others"


def _pinned_act_tables(arch):
    return {k: (v if k == _ACT_SET else set())
            for k, v in _orig_act_tables(arch).items()}


_bacc_mod.get_activation_tables = _pinned_act_tables

# The Tile scheduler orders each engine's queue with a cost model that
# assumes 360GB/s DMA. With 8 cores streaming concurrently a core
# really gets ~210GB/s, so the scheduler parks DMA-gated ops ahead of
# already-ready compute and the in-order engine queues stall on arrival
# (observed: a q-chunk Square scheduled between a rsqrt's Ln and Exp,
# blocking ACT ~6us). Feeding the scheduler the measured rate fixes the
# ordering. These class attrs are re-read at every schedule invocation.
import concourse.hw_specs as _hw_specs

_hw_specs.TRN2Spec.DMA_BUS_BYTES_PER_NS_PER_ENGINE = 210e9 / 16 / 1e9
_hw_specs.TRN2Spec.DMA_CYCLE = 1e9 / (210e9 / 128)

F32 = mybir.dt.float32
BF = mybir.dt.bfloat16
AX = mybir.AxisListType
OP = mybir.AluOpType
AF = mybir.ActivationFunctionType

# Problem constants (fixed by the problem statement).
B, QN, WAY, SHOT, HH, WW, C = 32, 75, 5, 5, 5, 5, 640
NCORES = 8
E = B // NCORES        # 4 episodes per core
HW = HH * WW           # 25 spatial positions
QD = QN * HW           # 1875 query descriptors / episode
SD = WAY * SHOT * HW   # 625 support descriptors / episode
P = 125                # descriptors per tile
QT = QD // P           # 15 query slots / episode (desc d = 15p + j)
ST = SD // P           # 5 support slots / episode (desc d = 5p + j)
NMAP = WAY * SHOT      # 25 support maps / episode
NCH = 3                # q DMA chunks (5 slots each)
SPC = QT // NCH        # slots per chunk
GAMMA = 5.0
SLOPE = 0.01
CH = C // 2            # 320-column halves (one PSUM bank each)
MB = 32                # mean-row base partition in the fused support psum
SW = MB + NMAP         # fused stationary width (57)
QNP = QN + 1           # padded transpose chunk stride (PSUM 4B align)

# constants tensor layout (free-axis offsets, bf16)
QM0 = 0                    # qmasks  [125, 15*75], value 1/25
SM0 = QM0 + QT * QN        # smasks  [125, 5*25],  value 1.0 (sums + att)
SM2 = SM0 + ST * NMAP      # smasks  [125, 5*25],  value 1/25 (hw-mean)
WB0 = SM2 + ST * NMAP      # w bcast [128, 640]
ID0 = WB0 + C              # identity [75, 75]
SH0 = ID0 + QN             # shotm   [25, 5], value 1/5
CW = SH0 + WAY             # = 2095

# engine split of the per-slot norm passes (True -> ACT)
S_ACT = (True, True, True, False, False)
Q_ACT = (True, True, True, True, False,
         True, True, True, False, False,
         True, True, False, False, False)


def _build_body(ctx: ExitStack, tc: "tile.TileContext", i1, i2, cst, out):
    nc = tc.nc

    cpool = ctx.enter_context(tc.tile_pool(name="consts", bufs=1))
    dpool = ctx.enter_context(tc.tile_pool(name="data", bufs=1))
    scr_pool = ctx.enter_context(tc.tile_pool(name="scratch", bufs=1))
    stats = ctx.enter_context(tc.tile_pool(name="stats", bufs=2))
    sel_pool = ctx.enter_context(tc.tile_pool(name="sel", bufs=2))
    sb_pool = ctx.enter_context(tc.tile_pool(name="sbwork", bufs=2))
    ps = ctx.enter_context(tc.tile_pool(name="ps", bufs=1, space="PSUM"))

    # ---- all input DMAs first (SWDGE; gpsimd queue head) ----
    # s_sl[e][j] is the [125, 640] slot view; episode 0's support tensor is
    # loaded as 5 per-slot DMAs so the very first compute tile lands ~10us
    # earlier (a big DMA's completion waits on all 125 descriptors).
    s_sl, q_t = [], []
    for e in range(E):
        if e == 0:
            sl = []
            for j in range(ST):
                t_ = dpool.tile([P, C], BF, name=f"s0_{j}", tag=f"s0_{j}")
                nc.gpsimd.dma_start(t_[:], i2[0, :, C * j:C * (j + 1)])
                sl.append(t_[:])
        else:
            st_ = dpool.tile([P, ST * C], BF, name=f"s_{e}", tag=f"s_{e}")
            nc.gpsimd.dma_start(st_[:], i2[e])
            sl = [st_[:, C * j:C * (j + 1)] for j in range(ST)]
        qc = []
        for c in range(NCH):
            if e == 0:
                # episode 0's chunks land per-slot so the q-norm pipeline
                # never stalls on a whole-chunk completion (~7us each)
                qsl = []
                for jj in range(SPC):
                    t_ = dpool.tile([P, C], BF, name=f"q0{c}_{jj}",
                                    tag=f"q0{c}_{jj}")
                    nc.gpsimd.dma_start(
                        t_[:], i1[0, :, C * (SPC * c + jj):
                                   C * (SPC * c + jj + 1)])
                    qsl.append(t_[:])
                qc.append(qsl)
            else:
                qt_ = dpool.tile([P, SPC * C], BF, name=f"q_{e}_{c}",
                                 tag=f"q_{e}_{c}")
                nc.gpsimd.dma_start(qt_[:],
                                    i1[e, :, SPC * C * c:SPC * C * (c + 1)])
                qc.append([qt_[:, C * jj:C * (jj + 1)] for jj in range(SPC)])
        s_sl.append(sl)
        q_t.append(qc)

    # ---- constants (host-precomputed, one sync DMA) ----
    consts = cpool.tile([128, CW], BF, name="consts")
    nc.sync.dma_start(consts[:], cst)
    smask = [consts[0:P, SM0 + NMAP * j:SM0 + NMAP * (j + 1)] for j in range(ST)]
    smask3 = consts[0:P, SM0:SM0 + ST * NMAP].rearrange(
        "p (j m) -> p j m", j=ST)
    smask3m = consts[0:P, SM2:SM2 + ST * NMAP].rearrange(
        "p (j m) -> p j m", j=ST)
    qmask3 = [consts[0:P, QM0 + SPC * QN * c:QM0 + SPC * QN * (c + 1)]
              .rearrange("p (j q) -> p j q", j=SPC) for c in range(NCH)]
    wbc = consts[0:P, WB0:WB0 + C]
    ident = consts[0:QN, ID0:ID0 + QN]
    shotm = consts[0:NMAP, SH0:SH0 + WAY]

    # fused support stationary [125, ST, 57] (cols 25-31 stay zero forever)
    st_all = cpool.tile([P, ST, SW], BF, name="st_all")
    nc.vector.memset(st_all[:, :, NMAP:MB], 0.0)

    def slot(big, j):
        return big[:, C * j:C * (j + 1)]

    def pe_tickle(anchor_ap):
        """Dummy 1-column LDWEIGHTS anchored to a freshly-written bf16
        tile. Executes in ~85ns as soon as the anchor is ready, keeping
        the PE HAM activity monitor from re-throttling the clock to
        1.2GHz during ACT/DVE-bound phases (idle windows >3.4us drop the
        PE to K=4/8). All real matmuls self-load weights, so clobbering
        the stationary register is safe."""
        nc.tensor.ldweights(anchor_ap)

    def rsqrt_act(dst, x, n, tag):
        """dst = 1/sqrt(x) on ACT: exp(-0.5*ln(x)); same table set as
        Square/Exp, so no ACT_TABLE_LOAD switches."""
        t = stats.tile([P, n], F32, name=f"rs_{tag}", tag=f"rs_{tag}")
        nc.scalar.activation(t[:], x, AF.Ln)
        nc.scalar.activation(dst, t[:], AF.Exp, scale=-0.5)

    def norm_pass(sl, acc_col, on_act):
        if on_act:
            scr = scr_pool.tile([P, C], BF, name="sq_a", tag="sq_a")
            nc.scalar.activation(scr[:], sl, AF.Square, accum_out=acc_col)
        else:
            scr = scr_pool.tile([P, C], BF, name="sq_v", tag="sq_v")
            nc.vector.scalar_tensor_tensor(
                out=scr[:], in0=sl, scalar=1.0, in1=sl,
                op0=OP.mult, op1=OP.mult, accum_out=acc_col)
        return scr

    # per-episode state carried into the pipelined tail
    qm_sb_t, ftT_t, tq_t, qmT_t = [None] * E, [None] * E, [None] * E, [None] * E

    def emit_tail_a(e):
        """PE transpose of qm (needs qm_sb[e]), on the prior episode's
        psum bank."""
        tq_ps = ps.tile([128, WAY * QNP], BF, name=f"tq_{e}", tag="tq")
        for cc in range(WAY):
            nc.tensor.transpose(tq_ps[:, QNP * cc:QNP * cc + QN],
                                qm_sb_t[e][:, 128 * cc:128 * (cc + 1)], ident)
        tq_t[e] = tq_ps
        qmT = sb_pool.tile([128, WAY * QNP], BF, name=f"qmT_{e}", tag="qmT")
        nc.scalar.copy(qmT[:], tq_ps[:])
        qmT_t[e] = qmT

    def emit_tail_b(e):
        sim_ps = ps.tile([WAY, QN], F32, name=f"sim_{e}", tag="sim")
        for cc in range(WAY):
            nc.tensor.matmul(sim_ps[:], ftT_t[e][:, WAY * cc:WAY * (cc + 1)],
                             qmT_t[e][:, QNP * cc:QNP * cc + QN],
                             start=(cc == 0), stop=(cc == WAY - 1))
        sim_sb = sb_pool.tile([WAY, QN], F32, name=f"sim_sb_{e}", tag="sim_sb")
        nc.vector.tensor_copy(sim_sb[:], sim_ps[:])
        nc.sync.dma_start(out[e], sim_sb[:])

    for e in range(E):
        ssl = s_sl[e]
        # ================= support side =================
        sn2 = stats.tile([P, ST], F32, name=f"sn2_{e}", tag="sn2")
        rr = stats.tile([P, ST], F32, name=f"rr_{e}", tag="rr")
        # DVE s-norms first so ACT's rsqrt isn't stuck behind the logits
        for j in range(ST):
            if not S_ACT[j]:
                norm_pass(ssl[j], sn2[:, j:j + 1], False)
        for j in range(ST):
            if S_ACT[j]:
                norm_pass(ssl[j], sn2[:, j:j + 1], True)
        for j in range(ST):
            scr2 = scr_pool.tile([P, C], BF, name="s_tt", tag="s_tt")
            nc.vector.scalar_tensor_tensor(
                out=scr2[:], in0=ssl[j], scalar=1.0, in1=wbc,
                op0=OP.mult, op1=OP.mult, accum_out=rr[:, j:j + 1])
        sinv = stats.tile([P, ST], BF, name=f"sinv_{e}", tag="sinv")
        rsqrt_act(sinv[:], sn2[:], ST, f"s{e % 2}")
        # softmax over hw within each map (logits tiny: no max-shift)
        lg = stats.tile([P, ST], F32, name=f"lg_{e}", tag="lg")
        nc.vector.tensor_mul(lg[:], rr[:], sinv[:])
        el = stats.tile([P, ST], BF, name=f"el_{e}", tag="el")
        nc.scalar.activation(el[:], lg[:], AF.Exp)
        # per-map sums of exp -> softmax reciprocal
        sums = ps.tile([NMAP, 1], F32, name=f"sums_{e}", tag="sums")
        for j in range(ST):
            nc.tensor.matmul(sums[:], smask[j], el[:, j:j + 1],
                             start=(j == 0), stop=(j == ST - 1))
        rec = stats.tile([NMAP, 1], F32, name=f"rec_{e}", tag="rec")
        nc.vector.reciprocal(rec[:], sums[:])
        # unnormalized att weights (softmax recip applied at cg evacuation)
        uw = stats.tile([P, ST], BF, name=f"uw_{e}", tag="uw")
        nc.vector.tensor_mul(uw[:], el[:], sinv[:])
        # batched stationary builds (pad cols stay zero)
        nc.vector.tensor_mul(st_all[:, :, 0:NMAP], smask3,
                             uw[:].broadcast_to((P, ST, NMAP)))
        nc.vector.tensor_mul(st_all[:, :, MB:SW], smask3m,
                             sinv[:].broadcast_to((P, ST, NMAP)))
        cg_ps = [ps.tile([SW, CH], F32, name=f"cg{h}_{e}", tag=f"cg{h}")
                 for h in range(2)]
        for j in range(ST):
            for h in range(2):
                nc.tensor.matmul(cg_ps[h][:], st_all[:, j, :],
                                 ssl[j][:, CH * h:CH * (h + 1)],
                                 start=(j == 0), stop=(j == ST - 1))
        # evacuate: cg rows 0-24 (x softmax recip), mean rows MB..MB+24
        cg_sb = sb_pool.tile([NMAP, C], F32, name=f"cg_sb_{e}", tag="cg_sb")
        for h in range(2):
            nc.vector.tensor_scalar_mul(cg_sb[:, CH * h:CH * (h + 1)],
                                        cg_ps[h][0:NMAP, :], rec[:, 0:1])
        lk = sb_pool.tile([NMAP, C], F32, name=f"lk_{e}", tag="lk")
        nc.vector.scalar_tensor_tensor(
            out=lk[:], in0=cg_sb[:], scalar=SLOPE, in1=cg_sb[:],
            op0=OP.mult, op1=OP.max)
        fp = sb_pool.tile([NMAP, C], BF, name=f"fp_{e}", tag="fp")
        for h in range(2):
            nc.vector.scalar_tensor_tensor(
                out=fp[:, CH * h:CH * (h + 1)], in0=lk[:, CH * h:CH * (h + 1)],
                scalar=GAMMA, in1=cg_ps[h][MB:MB + NMAP, :],
                op0=OP.mult, op1=OP.add)
        # featT[c, w] directly: fp.T @ shotm, chunked over c
        ftT_ps = ps.tile([128, WAY * WAY], F32, name=f"ftT_{e}", tag="ftT")
        for cc in range(WAY):
            nc.tensor.matmul(ftT_ps[:, WAY * cc:WAY * (cc + 1)],
                             fp[:, 128 * cc:128 * (cc + 1)], shotm)
        ftT = sb_pool.tile([128, WAY * WAY], BF, name=f"ftTs_{e}", tag="ftTs")
        nc.vector.tensor_copy(ftT[:], ftT_ps[:])
        ftT_t[e] = ftT

        # ================= query side =================
        qn2 = stats.tile([P, QT], F32, name=f"qn2_{e}", tag="qn2")
        qinv = stats.tile([P, QT], BF, name=f"qinv_{e}", tag="qinv")
        qm = [ps.tile([QN, CH], F32, name=f"qm{h}_{e}", tag=f"qm{h}")
              for h in range(2)]
        for c in range(NCH):
            for jj in range(SPC):
                j = SPC * c + jj
                norm_pass(q_t[e][c][jj], qn2[:, j:j + 1], Q_ACT[j])
            lo = SPC * c
            rsqrt_act(qinv[:, lo:lo + SPC], qn2[:, lo:lo + SPC], SPC,
                      f"q{c}_{e % 2}")
            sel = sel_pool.tile([P, SPC, QN], BF, name=f"sel{c}",
                                tag=f"sel{c}")
            nc.vector.tensor_mul(sel[:], qmask3[c],
                                 qinv[:, lo:lo + SPC]
                                 .broadcast_to((P, SPC, QN)))
            for jj in range(SPC):
                j = lo + jj
                for h in range(2):
                    nc.tensor.matmul(
                        qm[h][:], sel[:, jj, :],
                        q_t[e][c][jj][:, CH * h:CH * (h + 1)],
                        start=(j == 0), stop=(j == QT - 1))
            # spread the previous episode's tail into the q-norm gaps
            if e > 0:
                if c == 0:
                    emit_tail_a(e - 1)
                elif c == 1:
                    emit_tail_b(e - 1)
        qm_sb = sb_pool.tile([QN, C], BF, name=f"qm_sb_{e}", tag="qm_sb")
        for h in range(2):
            nc.scalar.copy(qm_sb[:, CH * h:CH * (h + 1)], qm[h][:])
        qm_sb_t[e] = qm_sb
    emit_tail_a(E - 1)
    emit_tail_b(E - 1)


def build_program():
    nc = bacc.Bacc("TRN2", target_bir_lowering=False, debug=False,
                   num_devices=NCORES)
    inp1 = nc.dram_tensor("input1", [E, P, QT * C], BF, kind="ExternalInput")
    inp2 = nc.dram_tensor("input2", [E, P, ST * C], BF, kind="ExternalInput")
    cst = nc.dram_tensor("consts", [128, CW], BF, kind="ExternalInput")
    out = nc.dram_tensor("sim", [E, WAY, QN], F32, kind="ExternalOutput")
    with tile.TileContext(nc) as tc, ExitStack() as ctx:
        _build_body(ctx, tc, inp1.ap(), inp2.ap(), cst.ap(), out.ap())
    nc.compile()
    return nc


_NC = None


def _get_nc():
    global _NC
    if _NC is None:
        _NC = build_program()
    return _NC


def _build_consts(rpn_w):
    cst = np.zeros((128, CW), np.float32)
    # qmask: descriptor d = 15p + j belongs to query q = d // 25
    pp = np.arange(P)
    for j in range(QT):
        cst[pp, QM0 + QN * j + (15 * pp + j) // HW] = 1.0 / HW
    for j in range(ST):
        cst[pp, SM0 + NMAP * j + (5 * pp + j) // HW] = 1.0
        cst[pp, SM2 + NMAP * j + (5 * pp + j) // HW] = 1.0 / HW
    cst[:, WB0:WB0 + C] = np.asarray(rpn_w, np.float32).reshape(1, C)
    cst[np.arange(QN), ID0 + np.arange(QN)] = 1.0
    m = np.arange(NMAP)
    cst[m, SH0 + m // SHOT] = 1.0 / SHOT
    return cst.astype(ml_dtypes.bfloat16)


def shard_inputs(input1, input2, rpn_w, rpn_b=None):
    """Shard over episodes; [E, 1875, 640] -> [E, 125, 15*640] is a pure
    reshape (descriptor d = 15p + j, slots consecutive in DRAM)."""
    i1 = np.asarray(input1, np.float32).reshape(B, P, QT * C).astype(
        ml_dtypes.bfloat16)
    i2 = np.asarray(input2, np.float32).reshape(B, P, ST * C).astype(
        ml_dtypes.bfloat16)
    cst = _build_consts(rpn_w)
    in_maps = []
    for i in range(NCORES):
        in_maps.append({
            "input1": np.ascontiguousarray(i1[E * i:E * (i + 1)]),
            "input2": np.ascontiguousarray(i2[E * i:E * (i + 1)]),
            "consts": cst,
        })
    return in_maps


def _ensure_ntff_hook():
    """Install the NTFF profile hook (the image's antenv lacks axon_hooks)."""
    import types
    import antenv

    if "antenv.axon_hooks" not in sys.modules:
        mod = types.ModuleType("antenv.axon_hooks")
        mod._hook = None
        mod.set_axon_ntff_profile_hook = lambda h: setattr(mod, "_hook", h)
        mod.get_axon_ntff_profile_hook = lambda: mod._hook
        sys.modules["antenv.axon_hooks"] = mod
        antenv.axon_hooks = mod
    mod = sys.modules["antenv.axon_hooks"]
    if mod.get_axon_ntff_profile_hook() is None:
        from trn_agent_boot.trn_boot import _ntff_profile_via_ctypes
        hook = _ntff_profile_via_ctypes("/opt/axon/libaxon_pjrt.so")
        if hook is not None:
            mod.set_axon_ntff_profile_hook(hook)


def kernel(input1, input2, rpn_w, rpn_b=None, **run_kwargs):
    if run_kwargs.get("trace"):
        _ensure_ntff_hook()
    nc = _get_nc()
    in_maps = shard_inputs(input1, input2, rpn_w)
    res = run_bass_kernel_spmd(nc, in_maps, list(range(NCORES)), **run_kwargs)
    # sim comes back [E, way, qn]; un-transpose on the host
    out = np.concatenate(
        [np.transpose(r["sim"], (0, 2, 1)) for r in res.results], axis=0)
    if run_kwargs:
        kernel.last_results = res
    return out.astype(np.float32)
